# revision 1
# baseline (speedup 1.0000x reference)
"""Bezier stroke renderer on 8 Trainium2 NeuronCores (Bass/Tile SPMD kernel).

Reference semantics: 32 cubic-Bezier strokes, each sampled into a 16-segment
polyline, rasterized onto a 1024x1024 canvas: per pixel and segment,
darkness = clip((2t - dist_to_segment)/(2t), 0, 1), max over segments within a
stroke, then grid = max(grid, darkness * color) over strokes (3 channels).

Strategy (sharding: spatial split of the pixel grid by rows):
  - The canvas is split into 16 blocks of 64 rows; each core owns 2 blocks
    (greedy-balanced), giving a [128 partitions x 1024 cols] canvas tile.
  - Only pixels within 2t+1 of a segment can be painted.  Host code builds
    (segment, block) column windows and interval-packs them into D0
    canvas-aligned depth slots; windows that do not fit become overflow
    items (1-2 chunks of 32 columns), placed FIRST in the packed axis so
    their scatters overlap the slot computation.  All per-column parameters
    ship as per-core tables; the SPMD instruction stream is identical on
    every core (overflow counts padded per width-class to the max).
  - Distance math in the segment's tangent frame, pre-scaled by 1/(2t):
        dist/(2t) = sqrt(relu(a-L)^2 + relu(-a)^2 + b^2)
    with a,b affine in pixel coords.  Coefficients are shipped as exact
    3-way fp16 splits so TensorE runs single-pass K=6 fp16 matmuls
    (fp32 PSUM).  The two overshoot relus are mutually exclusive, so
    (q1+q2)^2 = q1^2+q2^2 saves a square: ACT does relu/relu/square/
    square/sqrt, GPSIMD the two adds, DVE the fused (dd-1)*col_c into a
    channel-interleaved packed buffer vint.
  - Composite via two independent accumulators: overflow windows are
    min-scattered at register-loaded dynamic offsets into zero-init acc
    (the only dynamic-AP target, keeping Tile dependencies precise),
    while the D0 slots min-merge into accb with static APs as each
    slot's chunks finish; a piecewise combine/relu(-x)/DMA tail stores
    the result.  The host reassembles block rows into (3, 1024, 1024).
"""

import sys
import types
import contextlib
import ctypes

sys.path.insert(0, "/opt/trn_rl_repo")

import numpy as np

G = 1024
P = 16
N = 32
N_CORES = 8
BH = 64           # block height (rows)
NB = G // BH      # 16 blocks
BLOCKS_PER_CORE = NB // N_CORES
W_ITEM = 32       # columns per packed chunk-item
MAX_CLASS = 2     # overflow scatter windows are 1..MAX_CLASS chunk-items wide
D0 = 5            # canvas-aligned depth slots (scatter-free compositing)
CHUNK = 512       # packed columns per matmul/PSUM chunk
ITEMS_PER_CHUNK = CHUNK // W_ITEM  # 16

_PROG_CACHE = {}
_HOOK_INSTALLED = False


def _install_ntff_hook():
    """Register the NTFF profile hook (mirrors trn_boot.py) so
    run_bass_kernel_spmd(trace=True) can measure HW exec time."""
    global _HOOK_INSTALLED
    if _HOOK_INSTALLED:
        return
    _HOOK_INSTALLED = True
    try:
        import antenv
        mod = types.ModuleType("antenv.axon_hooks")
        holder = [None]
        mod.set_axon_ntff_profile_hook = lambda h: holder.__setitem__(0, h)
        mod.get_axon_ntff_profile_hook = lambda: holder[0]
        sys.modules["antenv.axon_hooks"] = mod
        antenv.axon_hooks = mod

        lib = ctypes.CDLL("/opt/axon/libaxon_pjrt.so")
        if not hasattr(lib, "axon_start_nrt_profile"):
            return
        lib.axon_start_nrt_profile.argtypes = [
            ctypes.POINTER(ctypes.c_int64),
            ctypes.c_size_t,
        ]
        lib.axon_start_nrt_profile.restype = ctypes.c_int64
        lib.axon_stop_nrt_profile.argtypes = [ctypes.c_char_p]
        lib.axon_stop_nrt_profile.restype = ctypes.c_int64

        @contextlib.contextmanager
        def _hook(output_dir, device_ids):
            import jax
            jax.devices()
            if device_ids:
                ids = (ctypes.c_int64 * len(device_ids))(*device_ids)
                rc = lib.axon_start_nrt_profile(ids, len(device_ids))
            else:
                rc = lib.axon_start_nrt_profile(None, 0)
            if rc != 0:
                raise RuntimeError(f"axon_start_nrt_profile rc={rc}")
            try:
                yield
            finally:
                n = lib.axon_stop_nrt_profile(str(output_dir).encode())
                print(f"profile: {n} file(s) written to {output_dir}",
                      file=sys.stderr)

        mod.set_axon_ntff_profile_hook(_hook)
    except Exception:
        pass


# ---------------------------------------------------------------- host side

def _bezier_weights_f32(p):
    t = np.arange(p, dtype=np.float64)
    w1 = (p - t) ** 3 / p ** 3
    w2 = 3 * (p - t) ** 2 * t / p ** 3
    w3 = 3 * (p - t) * t ** 2 / p ** 3
    w4 = t ** 3 / p ** 3
    return np.stack([w1, w2, w3, w4]).astype(np.float32)  # (4, P)


def _polylines(strokes):
    """(N,2,4) f32 -> (N, P+1, 2) f32 polyline points in pixel units,
    mirroring reference.curve_to_stroke in float32."""
    W = _bezier_weights_f32(P)
    s = strokes.astype(np.float32)
    pts, derivs = s[:, :, :2], s[:, :, 2:]
    before = pts - derivs
    after = pts + derivs
    p1, p2, p3, p4 = pts[:, :-1], after[:, :-1], before[:, 1:], pts[:, 1:]
    cp = np.stack([p1, p2, p3, p4], axis=3)          # (N, 1, 2, 4)
    sp = np.einsum("nsdk,kp->nspd", cp, W).astype(np.float32)  # (N,1,P,2)
    sp = sp.reshape(s.shape[0], -1, 2)
    poly = np.concatenate([sp, pts[:, -1:, :]], axis=1).astype(np.float32)
    return poly * np.float32(G)


def _band_clip(v, w, pad, x0, x1):
    """Clip segment v->w (f64) to row band [x0-pad, x1+pad]; return padded,
    canvas-clamped column range [c0, c1] or None."""
    lo_x, hi_x = x0 - pad, x1 + pad
    dx = w[0] - v[0]
    if abs(dx) < 1e-12:
        if v[0] < lo_x or v[0] > hi_x:
            return None
        s0, s1 = 0.0, 1.0
    else:
        sa = (lo_x - v[0]) / dx
        sb = (hi_x - v[0]) / dx
        s0 = max(0.0, min(sa, sb))
        s1 = min(1.0, max(sa, sb))
        if s0 > s1:
            return None
    ya = v[1] + s0 * (w[1] - v[1])
    yb = v[1] + s1 * (w[1] - v[1])
    c0 = max(0.0, min(ya, yb) - pad)
    c1 = min(G - 1.0, max(ya, yb) + pad)
    if c1 < c0:
        return None
    return int(np.floor(c0)), int(np.ceil(c1))


def _build_worklists(strokes, thicknesses, colors):
    """Returns (blocks_of_core, windows_per_core, t, col); windows are raw
    (n, v, w, c0, c1) column spans per (segment, block)."""
    poly = _polylines(strokes).astype(np.float64)          # (N, P+1, 2)
    t = np.maximum(thicknesses.astype(np.float32) * np.float32(2.0)
                   + np.float32(0.5), np.float32(0.5))[:, 0]  # f32 (N,)
    col = np.clip(colors.astype(np.float32), 0.0, 1.0)     # (N, 3)
    r = 2.0 * t.astype(np.float64)
    pad = r + 1.0

    wins_by_block = [[] for _ in range(NB)]
    cost = np.zeros(NB)
    for n in range(N):
        for i in range(P):
            v = poly[n, i]
            w = poly[n, i + 1]
            for b in range(NB):
                clip = _band_clip(v, w, pad[n], BH * b, BH * b + BH - 1)
                if clip is None:
                    continue
                c0, c1 = clip
                wins_by_block[b].append((n, v, w, c0, c1))
                cost[b] += c1 - c0 + 1

    order = np.argsort(-cost)
    loads = np.zeros(N_CORES)
    blocks_of = [[] for _ in range(N_CORES)]
    for b in order:
        cands = [c for c in range(N_CORES) if len(blocks_of[c]) < BLOCKS_PER_CORE]
        c = min(cands, key=lambda c: loads[c])
        blocks_of[c].append(int(b))
        loads[c] += cost[b]
    for c in range(N_CORES):
        blocks_of[c].sort()

    windows_per_core = [
        [it for b in blocks_of[c] for it in wins_by_block[b]]
        for c in range(N_CORES)
    ]
    return blocks_of, windows_per_core, t, col


def _assign_slots(windows):
    """Greedy interval packing of raw windows into D0 canvas-aligned slots.
    Returns (slot_wins: list of per-slot window lists, overflow: list of
    (n, v, w, c0, nch) chunked overflow windows)."""
    occ = np.zeros((D0, G), bool)
    slot_wins = [[] for _ in range(D0)]
    overflow = []
    for win in sorted(windows, key=lambda x: -(x[4] - x[3])):
        n, v, w, c0, c1 = win
        placed = False
        for d in range(D0):
            if not occ[d, c0:c1 + 1].any():
                occ[d, c0:c1 + 1] = True
                slot_wins[d].append(win)
                placed = True
                break
        if placed:
            continue
        width = c1 - c0 + 1
        cstart = c0
        while width > 0:
            nch = min(MAX_CLASS, int(np.ceil(width / W_ITEM)))
            cc = max(0, min(cstart, G - W_ITEM * nch))
            overflow.append((n, v, w, cc, nch))
            cstart += W_ITEM * nch
            width -= W_ITEM * nch
    return slot_wins, overflow


def _build_tables(blocks_of, slot_wins_pc, ovf_pc, t, col, class_counts):
    """Per-core tables.  Packed layout: D0 canvas-aligned slots of G columns,
    then overflow chunk-items (class-sorted, padded to class_counts)."""
    novf = sum(cc * (ci + 1) for ci, cc in enumerate(class_counts))
    nwin = sum(class_counts)
    packw = D0 * G + novf * W_ITEM
    in_maps = []
    for c in range(N_CORES):
        vx = np.zeros(packw); vy = np.zeros(packw)
        wx = np.zeros(packw); wy = np.zeros(packw)
        i2t = np.full(packw, 1.0)
        cols = np.zeros((packw, 3))
        valid = np.zeros(packw, bool)
        base_s = novf * W_ITEM        # slots live after the overflow region
        ycol = np.zeros(packw)
        ycol[base_s:] = np.tile(np.arange(G, dtype=np.float64), D0)

        def put(pos, m, n, v, w):
            vx[pos:pos + m] = v[0]; vy[pos:pos + m] = v[1]
            wx[pos:pos + m] = w[0]; wy[pos:pos + m] = w[1]
            i2t[pos:pos + m] = 1.0 / (2.0 * np.float64(t[n]))
            cols[pos:pos + m] = col[n]
            valid[pos:pos + m] = True

        for d in range(D0):
            for (n, v, w, c0, c1) in slot_wins_pc[c][d]:
                put(base_s + d * G + c0, c1 - c0 + 1, n, v, w)

        by_class = [[] for _ in range(MAX_CLASS)]
        for win in ovf_pc[c]:
            by_class[win[4] - 1].append(win)
        offv = np.zeros(nwin, np.int64)
        widx = 0
        pos = 0
        for ci in range(MAX_CLASS):
            assert len(by_class[ci]) <= class_counts[ci]
            for k in range(class_counts[ci]):
                if k < len(by_class[ci]):
                    n, v, w, c0, nch = by_class[ci][k]
                    offv[widx] = 3 * c0
                    put(pos, W_ITEM * nch, n, v, w)
                    ycol[pos:pos + W_ITEM * nch] = \
                        c0 + np.arange(W_ITEM * nch, dtype=np.float64)
                widx += 1
                pos += W_ITEM * (ci + 1)
        assert pos == base_s and widx == nwin

        dx = wx - vx
        dy = wy - vy
        L = np.hypot(dx, dy)
        safe = L > 1e-9
        taux = np.where(safe, dx / np.where(safe, L, 1.0), 1.0)
        tauy = np.where(safe, dy / np.where(safe, L, 1.0), 0.0)
        Leff = np.where(safe, L, 0.0)
        nux = -tauy
        nuy = taux

        av = vx * taux + vy * tauy
        bv = vx * nux + vy * nuy
        a1 = taux * i2t                                   # x coef
        a2 = (ycol * tauy - av) * i2t                     # const (tangent)
        b1 = nux * i2t
        b2 = (ycol * nuy - bv) * i2t
        ll = Leff * i2t

        dead = ~valid
        a1[dead] = 0.0; a2[dead] = 0.0
        b1[dead] = 0.0; b2[dead] = 0.0; ll[dead] = 0.0
        cols[dead] = 0.0

        def split3(v):
            """Exact-ish 3-way fp16 split: v ~= h + m + l to ~2^-33 rel."""
            h = v.astype(np.float16)
            m = (v - h.astype(np.float64)).astype(np.float16)
            l = (v - h.astype(np.float64) - m.astype(np.float64)) \
                .astype(np.float16)
            return h, m, l

        # rt rows per family f: [const_h, const_m, const_l, x_h, x_m, x_l]
        # families: 0 = (a - L), 1 = a, 2 = b;  lhsT rows are (1,1,1,x,x,x)
        rt = np.zeros((18, packw), np.float16)
        for f, (const, xcoef) in enumerate(
                ((a2 - ll, a1), (a2, a1), (b2, b1))):
            rt[6 * f:6 * f + 3] = split3(const)
            rt[6 * f + 3:6 * f + 6] = split3(xcoef)
        # colors: 3-way split per channel, K=3 against the ones rows
        rc = np.zeros((9, packw), np.float16)
        for ch3 in range(3):
            rc[3 * ch3:3 * ch3 + 3] = split3(cols[:, ch3])
        off = offv.astype(np.int32).reshape(1, max(nwin, 1))

        xs = np.zeros(128, np.float64)
        for half, b in enumerate(blocks_of[c]):
            xs[half * BH:(half + 1) * BH] = BH * b + np.arange(BH)
        xt = np.zeros((70, 128), np.float16)
        for base in (0, 32, 64):
            xt[base:base + 3] = 1.0
            xt[base + 3:base + 6] = xs.astype(np.float16)  # exact (<2048)

        in_maps.append({"xt": xt, "rt": rt, "rc": rc, "off": off})
    return in_maps


# ---------------------------------------------------------------- bass side

def _build_program(class_counts):
    import concourse.bacc as bacc
    import concourse.mybir as mybir
    import concourse.bass as bass
    from concourse import tile

    f32 = mybir.dt.float32
    novf = sum(cc * (ci + 1) for ci, cc in enumerate(class_counts))
    nwin = sum(class_counts)
    packw = D0 * G + novf * W_ITEM
    nchunks = packw // CHUNK
    assert nchunks * CHUNK == packw

    nc = bacc.Bacc("TRN2", target_bir_lowering=False, debug=False,
                   num_devices=N_CORES)
    f16 = mybir.dt.float16
    xt_d = nc.dram_tensor("xt", [70, 128], f16, kind="ExternalInput").ap()
    rt_d = nc.dram_tensor("rt", [18, packw], f16, kind="ExternalInput").ap()
    rc_d = nc.dram_tensor("rc", [9, packw], f16, kind="ExternalInput").ap()
    off_d = nc.dram_tensor("off", [1, max(nwin, 1)], mybir.dt.int32,
                           kind="ExternalInput").ap()
    out_d = nc.dram_tensor("out", [128, 3 * G], f32, kind="ExternalOutput").ap()

    AF = mybir.ActivationFunctionType
    OP = mybir.AluOpType

    with tile.TileContext(nc) as tc:
        with (
            tc.tile_pool(name="const", bufs=1) as constp,
            tc.tile_pool(name="work", bufs=3) as workp,
            tc.tile_pool(name="psum", bufs=8, space="PSUM") as psump,
        ):
            # matmul operand pairs must sit at base partitions 0/32/64,
            # matching between lhsT and rhs
            xt = constp.tile([70, 128], f16)
            nc.sync.dma_start(xt[:], xt_d[:])
            rt = constp.tile([70, packw], f16)
            nc.sync.dma_start(rt[0:6, :], rt_d[0:6, :])
            nc.sync.dma_start(rt[32:38, :], rt_d[6:12, :])
            nc.sync.dma_start(rt[64:70, :], rt_d[12:18, :])
            rc = constp.tile([67, packw], f16)
            nc.sync.dma_start(rc[0:3, :], rc_d[0:3, :])
            nc.sync.dma_start(rc[32:35, :], rc_d[3:6, :])
            nc.sync.dma_start(rc[64:67, :], rc_d[6:9, :])
            off = constp.tile([1, max(nwin, 1)], mybir.dt.int32)
            nc.sync.dma_start(off[:], off_d[:])

            # vint: channel-interleaved packed values (c fastest).
            # Overflow chunk-items first (so their scatters can start while
            # the slot chunks are still computing), then D0 canvas slots.
            vint = constp.tile([128, 3 * packw], f32)
            vint3 = vint[:].rearrange("p (j c) -> p j c", c=3)
            base_s = 3 * novf * W_ITEM
            acc = constp.tile([128, 3 * G], f32)
            nc.gpsimd.memset(acc[:], 0.0)

            for ch in range(nchunks):
                sl = slice(ch * CHUNK, (ch + 1) * CHUNK)
                pal = psump.tile([128, CHUNK], f32, tag="ps")
                pa = psump.tile([128, CHUNK], f32, tag="ps")
                pb = psump.tile([128, CHUNK], f32, tag="ps")
                pc0 = psump.tile([128, CHUNK], f32, tag="ps")
                pc1 = psump.tile([128, CHUNK], f32, tag="ps")
                pc2 = psump.tile([128, CHUNK], f32, tag="ps")

                nc.tensor.matmul(pal[:], xt[0:6, :], rt[0:6, sl])
                nc.tensor.matmul(pa[:], xt[32:38, :], rt[32:38, sl])
                nc.tensor.matmul(pb[:], xt[64:70, :], rt[64:70, sl])
                nc.tensor.matmul(pc0[:], xt[0:3, :], rc[0:3, sl])
                nc.tensor.matmul(pc1[:], xt[32:35, :], rc[32:35, sl])
                nc.tensor.matmul(pc2[:], xt[64:67, :], rc[64:67, sl])

                q1 = workp.tile([128, CHUNK], f32, tag="q1")
                q2 = workp.tile([128, CHUNK], f32, tag="q2")
                o = workp.tile([128, CHUNK], f32, tag="o")
                so = workp.tile([128, CHUNK], f32, tag="so")
                sb = workp.tile([128, CHUNK], f32, tag="sb")
                d2 = workp.tile([128, CHUNK], f32, tag="d2")
                dd = workp.tile([128, CHUNK], f32, tag="dd")

                # overshoot beyond segment end / before start, in 2t units;
                # at most one of q1/q2 is nonzero, so (q1+q2)^2 = q1^2+q2^2
                nc.scalar.activation(q1[:], pal[:], AF.Relu)
                nc.scalar.activation(q2[:], pa[:], AF.Relu, scale=-1.0)
                nc.gpsimd.tensor_tensor(o[:], q1[:], q2[:], op=OP.add)
                nc.scalar.activation(so[:], o[:], AF.Square)
                nc.scalar.activation(sb[:], pb[:], AF.Square)
                nc.gpsimd.tensor_tensor(d2[:], so[:], sb[:], op=OP.add)
                nc.scalar.activation(dd[:], d2[:], AF.Sqrt)

                # w_c = (dd - 1) * col_c into channel-interleaved vint
                vch = vint3[:, sl, :]
                nc.vector.scalar_tensor_tensor(
                    vch[:, :, 0], dd[:], 1.0, pc0[:],
                    op0=OP.subtract, op1=OP.mult)
                nc.vector.scalar_tensor_tensor(
                    vch[:, :, 1], dd[:], 1.0, pc1[:],
                    op0=OP.subtract, op1=OP.mult)
                nc.vector.scalar_tensor_tensor(
                    vch[:, :, 2], dd[:], 1.0, pc2[:],
                    op0=OP.subtract, op1=OP.mult)

            # overflow scatter: min-composite window spans into acc
            # (emitted before the slot merges: overflow chunks are computed
            # first, so these fill DVE's early pipeline)
            BATCH = 8
            widx = 0
            pk = 0
            for ci in range(MAX_CLASS):
                wspan = 3 * W_ITEM * (ci + 1)
                cls_n = class_counts[ci]
                done = 0
                while done < cls_n:
                    cnt = min(BATCH, cls_n - done)
                    _, vals = nc.values_load_multi_w_load_instructions(
                        off[0:1, widx:widx + cnt],
                        engines=[nc.vector.engine],
                        min_val=0,
                        max_val=3 * (G - W_ITEM * (ci + 1)),
                        skip_runtime_bounds_check=True,
                    )
                    for val in vals:
                        dst = acc[:, bass.ds(val, wspan)]
                        src = vint[:, 3 * W_ITEM * pk:
                                   3 * W_ITEM * pk + wspan]
                        nc.vector.tensor_tensor(dst, dst, src, op=OP.min)
                        pk += ci + 1
                        widx += 1
                    done += cnt
            assert pk == novf and widx == nwin

            # composite the canvas-aligned slots into a SECOND accumulator
            # (independent of the scatter target, so slot merges pipeline
            # with compute as each slot's chunks finish instead of queueing
            # behind the scatter chain on acc)
            accb = constp.tile([128, 3 * G], f32)
            NPIECE = 2
            for piece in range(NPIECE):
                slp = slice(piece * 3 * G // NPIECE,
                            (piece + 1) * 3 * G // NPIECE)
                for d in range(D0):
                    ssl = slice(base_s + 3 * G * d + piece * 3 * G // NPIECE,
                                base_s + 3 * G * d +
                                (piece + 1) * 3 * G // NPIECE)
                    if d == 1:
                        # first write: min of slots 0 and 1 (no init needed)
                        s0l = slice(base_s + piece * 3 * G // NPIECE,
                                    base_s + (piece + 1) * 3 * G // NPIECE)
                        nc.vector.tensor_tensor(accb[:, slp], vint[:, s0l],
                                                vint[:, ssl], op=OP.min)
                    elif d > 1:
                        nc.vector.tensor_tensor(accb[:, slp], accb[:, slp],
                                                vint[:, ssl], op=OP.min)

            # combine, negate (with zero floor), store -- piecewise tail;
            # accb doubles as the negate staging buffer (dead after combine)
            for piece in range(4):
                slp = slice(piece * 3 * G // 4, (piece + 1) * 3 * G // 4)
                nc.vector.tensor_tensor(acc[:, slp], acc[:, slp],
                                        accb[:, slp], op=OP.min)
                nc.scalar.activation(accb[:, slp], acc[:, slp],
                                     AF.Relu, scale=-1.0)
                nc.sync.dma_start(out_d[:, slp], accb[:, slp])

    nc.compile()
    return nc


# ---------------------------------------------------------------- entry

def _prepare(strokes, thicknesses, colors):
    blocks_of, windows_per_core, t, col = _build_worklists(
        strokes, thicknesses, colors)
    slot_wins_pc = []
    ovf_pc = []
    class_counts = [0] * MAX_CLASS
    for c in range(N_CORES):
        sw, ovf = _assign_slots(windows_per_core[c])
        slot_wins_pc.append(sw)
        ovf_pc.append(ovf)
        per = [0] * MAX_CLASS
        for win in ovf:
            per[win[4] - 1] += 1
        for ci in range(MAX_CLASS):
            class_counts[ci] = max(class_counts[ci], per[ci])
    # pad class-1 count so total overflow chunk-items is a multiple of 16
    total = sum(cc * (ci + 1) for ci, cc in enumerate(class_counts))
    rem = (-total) % ITEMS_PER_CHUNK
    class_counts[0] += rem
    class_counts = tuple(class_counts)
    in_maps = _build_tables(blocks_of, slot_wins_pc, ovf_pc, t, col,
                            class_counts)
    return blocks_of, in_maps, class_counts


def kernel(strokes, thicknesses, colors):
    _install_ntff_hook()
    from concourse.bass_utils import run_bass_kernel_spmd

    strokes = np.asarray(strokes)
    thicknesses = np.asarray(thicknesses)
    colors = np.asarray(colors)

    blocks_of, in_maps, class_counts = _prepare(
        strokes, thicknesses, colors)
    if class_counts not in _PROG_CACHE:
        _PROG_CACHE[class_counts] = _build_program(class_counts)
    nc = _PROG_CACHE[class_counts]

    res = run_bass_kernel_spmd(nc, in_maps, list(range(N_CORES)))

    out = np.zeros((3, G, G), np.float32)
    for c in range(N_CORES):
        o = res.results[c]["out"].reshape(128, G, 3)     # (y, c) interleaved
        for half, b in enumerate(blocks_of[c]):
            rows = o[half * BH:(half + 1) * BH]          # (64, 1024, 3)
            out[:, BH * b:BH * (b + 1), :] = rows.transpose(2, 0, 1)
    return out


if __name__ == "__main__":
    rng = np.random.default_rng(0)
    s = rng.random((N, 2, 4), np.float32)
    th = rng.random((N, 1), np.float32)
    co = rng.random((N, 3), np.float32)
    g = kernel(s, th, co)
    print("out", g.shape, g.dtype, g.min(), g.max())



# revision 13
# speedup vs baseline: 1.1371x; 1.1371x over previous
"""Bezier stroke renderer on 8 Trainium2 NeuronCores (Bass/Tile SPMD kernel).

Reference semantics: 32 cubic-Bezier strokes, each sampled into a 16-segment
polyline, rasterized onto a 1024x1024 canvas: per pixel and segment,
darkness = clip((2t - dist_to_segment)/(2t), 0, 1), max over segments within a
stroke, then grid = max(grid, darkness * color) over strokes (3 channels).

Strategy (v3):
  - Canvas split into 16 blocks of 64 rows; block pairs assigned to cores by
    local-search balancing; a core's tile is [128 partitions x 1024 cols].
  - Per-column sweep packing: at each canvas column the (<= D0) active
    windows occupy canvas-aligned full-G slots (windows fragment freely at
    column granularity, so slot overflow is exactly max(0, depth-D0));
    excess runs become width-classed items (16/32/64/128 cols), composited
    with dynamic-offset min ops (register-loaded offsets, SPMD-identical
    instruction stream).
  - Distance math in the segment tangent frame, prescaled by 1/(2t):
    3 matmul families (a-L, a, b) with 2-way fp16 coefficient splits (K=4)
    into f32 PSUM superchunks of 1024 cols (2 banks per family).
  - Post-PSUM chain in bf16 (tolerance 2e-2 allows it):
      m = max(a-L, -a)        [gpsimd scalar_tensor_tensor from PSUM]
      o = max(m, 0)           [DVE tensor_scalar, 4x mode]
      so = o*o                [DVE tensor_tensor, 2x]
      sb = b*b                [ACT Square from PSUM]
      d2 = so + sb            [DVE tensor_tensor]
      dd = sqrt(d2)           [ACT]
      nd = dd - 1             [DVE tensor_scalar]
      v_c = nd * col_c        [DVE tensor_tensor vs a DMA-broadcast color
                               plane; planar per-superchunk layout]
  - Composite: acc[128, 3*G] bf16 (3 planes), min-composite: items scatter
    with 3-plane strided APs at dynamic offsets (split DVE/gpsimd), slots
    merge with static 3-plane min ops; tail = piecewise ACT relu(-x) -> f32
    + DMA out.
"""

import sys
import types
import contextlib
import ctypes

sys.path.insert(0, "/opt/trn_rl_repo")

import numpy as np
import ml_dtypes

G = 1024
P = 16
N = 32
N_CORES = 8
BH = 64             # block height (rows)
NB = G // BH        # 16 blocks
D0 = 4              # canvas-aligned slots
SUPER = 1024        # superchunk columns (2 PSUM banks per family)
ITEM_CLASSES = (16, 32, 64, 128)
SCATTER_BATCH = 8

_bf16 = ml_dtypes.bfloat16
_PROG_CACHE = {}
_HOOK_INSTALLED = False


def _install_ntff_hook():
    """Register the NTFF profile hook so run_bass_kernel_spmd(trace=True)
    can measure HW exec time."""
    global _HOOK_INSTALLED
    if _HOOK_INSTALLED:
        return
    _HOOK_INSTALLED = True
    try:
        import antenv
        mod = types.ModuleType("antenv.axon_hooks")
        holder = [None]
        mod.set_axon_ntff_profile_hook = lambda h: holder.__setitem__(0, h)
        mod.get_axon_ntff_profile_hook = lambda: holder[0]
        sys.modules["antenv.axon_hooks"] = mod
        antenv.axon_hooks = mod

        lib = ctypes.CDLL("/opt/axon/libaxon_pjrt.so")
        if not hasattr(lib, "axon_start_nrt_profile"):
            return
        lib.axon_start_nrt_profile.argtypes = [
            ctypes.POINTER(ctypes.c_int64),
            ctypes.c_size_t,
        ]
        lib.axon_start_nrt_profile.restype = ctypes.c_int64
        lib.axon_stop_nrt_profile.argtypes = [ctypes.c_char_p]
        lib.axon_stop_nrt_profile.restype = ctypes.c_int64

        @contextlib.contextmanager
        def _hook(output_dir, device_ids):
            import jax
            jax.devices()
            if device_ids:
                ids = (ctypes.c_int64 * len(device_ids))(*device_ids)
                rc = lib.axon_start_nrt_profile(ids, len(device_ids))
            else:
                rc = lib.axon_start_nrt_profile(None, 0)
            if rc != 0:
                raise RuntimeError(f"axon_start_nrt_profile rc={rc}")
            try:
                yield
            finally:
                n = lib.axon_stop_nrt_profile(str(output_dir).encode())
                print(f"profile: {n} file(s) written to {output_dir}",
                      file=sys.stderr)

        mod.set_axon_ntff_profile_hook(_hook)
    except Exception:
        pass


# ---------------------------------------------------------------- host side

def _bezier_weights_f32(p):
    t = np.arange(p, dtype=np.float64)
    w1 = (p - t) ** 3 / p ** 3
    w2 = 3 * (p - t) ** 2 * t / p ** 3
    w3 = 3 * (p - t) * t ** 2 / p ** 3
    w4 = t ** 3 / p ** 3
    return np.stack([w1, w2, w3, w4]).astype(np.float32)


def _polylines(strokes):
    W = _bezier_weights_f32(P)
    s = strokes.astype(np.float32)
    pts, derivs = s[:, :, :2], s[:, :, 2:]
    before = pts - derivs
    after = pts + derivs
    p1, p2, p3, p4 = pts[:, :-1], after[:, :-1], before[:, 1:], pts[:, 1:]
    cp = np.stack([p1, p2, p3, p4], axis=3)
    sp = np.einsum("nsdk,kp->nspd", cp, W).astype(np.float32)
    sp = sp.reshape(s.shape[0], -1, 2)
    poly = np.concatenate([sp, pts[:, -1:, :]], axis=1).astype(np.float32)
    return poly * np.float32(G)


def _band_clip(v, w, pad, x0, x1):
    lo_x, hi_x = x0 - pad, x1 + pad
    dx = w[0] - v[0]
    if abs(dx) < 1e-12:
        if v[0] < lo_x or v[0] > hi_x:
            return None
        s0, s1 = 0.0, 1.0
    else:
        sa = (lo_x - v[0]) / dx
        sb = (hi_x - v[0]) / dx
        s0 = max(0.0, min(sa, sb))
        s1 = min(1.0, max(sa, sb))
        if s0 > s1:
            return None
    ya = v[1] + s0 * (w[1] - v[1])
    yb = v[1] + s1 * (w[1] - v[1])
    c0 = max(0.0, min(ya, yb) - pad)
    c1 = min(G - 1.0, max(ya, yb) + pad)
    if c1 < c0:
        return None
    return int(np.floor(c0)), int(np.ceil(c1))


def _build_blocks(strokes, thicknesses, colors):
    poly = _polylines(strokes).astype(np.float64)
    t = np.maximum(thicknesses.astype(np.float32) * np.float32(2.0)
                   + np.float32(0.5), np.float32(0.5))[:, 0]
    col = np.clip(colors.astype(np.float32), 0.0, 1.0)
    pad = 2.0 * t.astype(np.float64) + 1.0
    wins_by_block = [[] for _ in range(NB)]
    for n in range(N):
        for i in range(P):
            v = poly[n, i]
            w = poly[n, i + 1]
            for b in range(NB):
                clip = _band_clip(v, w, pad[n], BH * b, BH * b + BH - 1)
                if clip is None:
                    continue
                c0, c1 = clip
                wins_by_block[b].append((n, v, w, c0, c1))
    return wins_by_block, t, col


def _depth_profile(wins):
    d = np.zeros(G, np.int64)
    for (_, _, _, c0, c1) in wins:
        d[c0:c1 + 1] += 1
    return d


def _pair_blocks(wins_by_block):
    """Pair blocks two-per-core, local-search minimizing per-core excess."""
    profs = [_depth_profile(wins_by_block[b]) for b in range(NB)]

    def cost(b1, b2):
        d = profs[b1] + profs[b2]
        return np.maximum(d - D0, 0).sum(), d.sum()

    order = sorted(range(NB),
                   key=lambda b: -np.maximum(profs[b] - D0, 0).sum())
    pairs = [[order[i], order[NB - 1 - i]] for i in range(NB // 2)]

    def metric(ps):
        cs = [cost(p[0], p[1]) for p in ps]
        return (max(c[0] for c in cs), sum(c[0] for c in cs),
                max(c[1] for c in cs))

    best = metric(pairs)
    improved = True
    while improved:
        improved = False
        for i in range(len(pairs)):
            for j in range(i + 1, len(pairs)):
                for a in range(2):
                    for b in range(2):
                        pairs[i][a], pairs[j][b] = pairs[j][b], pairs[i][a]
                        m = metric(pairs)
                        if m < best:
                            best = m
                            improved = True
                        else:
                            pairs[i][a], pairs[j][b] = pairs[j][b], pairs[i][a]
    return [sorted(p) for p in pairs]


def _sweep_pack(wins):
    """Per-column sweep: active windows (arrival order) -> slot levels;
    excess -> maximal runs. Returns (slots (D0,G) win-index or -1, runs)."""
    evs = sorted(range(len(wins)), key=lambda i: wins[i][3])
    slots = np.full((D0, G), -1, np.int64)
    active = []
    ei = 0
    run_open = {}
    runs = []
    for y in range(G):
        while ei < len(evs) and wins[evs[ei]][3] == y:
            active.append((evs[ei], wins[evs[ei]][4]))
            ei += 1
        active = [(i, c1) for (i, c1) in active if c1 >= y]
        excess_now = set()
        for lvl, (i, c1) in enumerate(active):
            if lvl < D0:
                slots[lvl, y] = i
            else:
                excess_now.add(i)
        for i in list(run_open):
            if i not in excess_now:
                runs.append((i, run_open.pop(i), y - 1))
        for i in excess_now:
            if i not in run_open:
                run_open[i] = y
    for i, c0 in run_open.items():
        runs.append((i, c0, G - 1))
    return slots, runs


def _runs_to_items(runs):
    items = []
    for (i, c0, c1) in runs:
        w = c1 - c0 + 1
        start = c0
        while w > 0:
            take = min(ITEM_CLASSES[-1], w)
            cls = min(c for c in ITEM_CLASSES if c >= take)
            items.append((i, max(0, min(start, G - cls)), cls))
            start += take
            w -= take
    return items


def _split2(v):
    h = v.astype(np.float16)
    l = (v - h.astype(np.float64)).astype(np.float16)
    return h, l


def _item_geometry(class_counts):
    """Packed (class_w, packed_pos) per item, never crossing a SUPER
    boundary; returns (geom, item_region_w) with item_region_w a multiple
    of nothing in particular (caller pads)."""
    geom = []
    pos = 0
    for cw, cnt in zip(ITEM_CLASSES, class_counts):
        for _ in range(cnt):
            if pos % SUPER + cw > SUPER:
                pos = (pos // SUPER + 1) * SUPER
            geom.append((cw, pos))
            pos += cw
    return geom, pos


def _layout(class_counts):
    geom, item_w = _item_geometry(class_counts)
    packw = -(-(item_w + D0 * G) // SUPER) * SUPER
    return geom, packw


def _build_tables(wins, slots, items, class_counts, t, col, blocks):
    geom, packw = _layout(class_counts)

    widx = np.full(packw, -1, np.int64)
    ycol = np.zeros(packw, np.float64)
    offs = []
    by_class = {cw: [] for cw in ITEM_CLASSES}
    for (i, c0, cls) in items:
        by_class[cls].append((i, c0))
    ki = 0
    for cw, cnt in zip(ITEM_CLASSES, class_counts):
        lst = by_class[cw]
        assert len(lst) <= cnt
        for k in range(cnt):
            cwg, pos = geom[ki]
            assert cwg == cw
            if k < len(lst):
                i, c0 = lst[k]
                widx[pos:pos + cw] = i
                ycol[pos:pos + cw] = c0 + np.arange(cw)
                offs.append(c0)
            else:
                offs.append(0)
            ki += 1
    pos = packw - D0 * G
    for d in range(D0):
        widx[pos:pos + G] = slots[d]
        ycol[pos:pos + G] = np.arange(G)
        pos += G
    assert pos == packw

    nw = len(wins)
    vx = np.array([w[1][0] for w in wins] + [0.0])
    vy = np.array([w[1][1] for w in wins] + [0.0])
    wx = np.array([w[2][0] for w in wins] + [0.0])
    wy = np.array([w[2][1] for w in wins] + [0.0])
    tn = np.array([t[w[0]] for w in wins] + [1.0], np.float64)
    cn = np.array([col[w[0]] for w in wins] + [[0.0, 0.0, 0.0]], np.float64)

    wi = np.where(widx < 0, nw, widx)
    dead = widx < 0
    i2t = 1.0 / (2.0 * tn[wi])
    dx = wx[wi] - vx[wi]
    dy = wy[wi] - vy[wi]
    L = np.hypot(dx, dy)
    safe = L > 1e-9
    taux = np.where(safe, dx / np.where(safe, L, 1.0), 1.0)
    tauy = np.where(safe, dy / np.where(safe, L, 1.0), 0.0)
    Leff = np.where(safe, L, 0.0)
    nux, nuy = -tauy, taux
    av = vx[wi] * taux + vy[wi] * tauy
    bv = vx[wi] * nux + vy[wi] * nuy
    a1 = taux * i2t
    a2 = (ycol * tauy - av) * i2t
    b1 = nux * i2t
    b2 = (ycol * nuy - bv) * i2t
    ll = Leff * i2t
    for arr in (a1, a2, b1, b2, ll):
        arr[dead] = 0.0
    colp = cn[wi].T.copy()
    colp[:, dead] = 0.0

    rt = np.zeros((12, packw), np.float16)
    for f, (const, xc) in enumerate(((a2 - ll, a1), (a2, a1), (b2, b1))):
        ch, cl = _split2(const)
        xh, xl = _split2(xc)
        rt[4 * f + 0] = ch
        rt[4 * f + 1] = cl
        rt[4 * f + 2] = xh
        rt[4 * f + 3] = xl

    # colb: planar per superchunk [c0-plane | c1-plane | c2-plane] x nsuper,
    # broadcast to 128 partitions
    nsuper = packw // SUPER
    colrow = np.zeros(3 * packw, np.float64)
    for s in range(nsuper):
        for c in range(3):
            colrow[3 * SUPER * s + c * SUPER:
                   3 * SUPER * s + (c + 1) * SUPER] = \
                colp[c, SUPER * s:SUPER * (s + 1)]
    colb = np.broadcast_to(colrow.astype(_bf16), (128, 3 * packw))
    colb = np.ascontiguousarray(colb)

    xs = np.zeros(128, np.float64)
    for half, b in enumerate(blocks):
        xs[half * BH:(half + 1) * BH] = BH * b + np.arange(BH)
    xt = np.zeros((68, 128), np.float16)
    for base in (0, 32, 64):
        xt[base + 0:base + 2] = 1.0
        xt[base + 2:base + 4] = xs.astype(np.float16)  # exact (< 2048)

    off = np.array(offs or [0], np.int32).reshape(1, -1)
    return dict(xt=xt, rt=rt, colb=colb, off=off), packw


# ---------------------------------------------------------------- bass side

def _build_program(class_counts, packw):
    import concourse.bacc as bacc
    import concourse.mybir as mybir
    import concourse.bass as bass
    from concourse import tile

    f32 = mybir.dt.float32
    f16 = mybir.dt.float16
    bf16 = mybir.dt.bfloat16
    i32 = mybir.dt.int32
    AF = mybir.ActivationFunctionType
    OP = mybir.AluOpType

    nitems = sum(class_counts)
    geom, packw2 = _layout(class_counts)
    assert packw2 == packw
    nsuper = packw // SUPER

    nc = bacc.Bacc("TRN2", target_bir_lowering=False, debug=False,
                   num_devices=N_CORES)
    xt_d = nc.dram_tensor("xt", [68, 128], f16, kind="ExternalInput").ap()
    rt_d = nc.dram_tensor("rt", [12, packw], f16, kind="ExternalInput").ap()
    colb_d = nc.dram_tensor("colb", [128, 3 * packw], bf16,
                            kind="ExternalInput").ap()
    off_d = nc.dram_tensor("off", [1, max(nitems, 1)], i32,
                           kind="ExternalInput").ap()
    out_d = nc.dram_tensor("out", [128, 3 * G], f32, kind="ExternalOutput").ap()

    with tile.TileContext(nc) as tc:
        with (
            tc.tile_pool(name="const", bufs=1) as constp,
            tc.tile_pool(name="work", bufs=3) as workp,
            tc.tile_pool(name="psum", bufs=4, space="PSUM") as psump,
        ):
            xt = constp.tile([68, 128], f16)
            nc.sync.dma_start(xt[:], xt_d[:])
            rt = constp.tile([68, packw], f16)
            nc.sync.dma_start(rt[0:4, :], rt_d[0:4, :])
            nc.sync.dma_start(rt[32:36, :], rt_d[4:8, :])
            nc.sync.dma_start(rt[64:68, :], rt_d[8:12, :])
            off = constp.tile([1, max(nitems, 1)], i32)
            nc.sync.dma_start(off[:], off_d[:])
            colb = constp.tile([128, 3 * packw], bf16)
            for s in range(nsuper):
                sl3 = slice(3 * SUPER * s, 3 * SUPER * (s + 1))
                nc.sync.dma_start(colb[:, sl3], colb_d[:, sl3])

            vint = constp.tile([128, 3 * packw], bf16)
            acc = constp.tile([128, 3 * G], bf16)
            nc.gpsimd.memset(acc[:], 0.0)

            acc3 = acc[:].rearrange("p (c g) -> p c g", c=3)
            vint4 = vint[:].rearrange("p (s c g) -> p s c g", s=nsuper, c=3)

            # register-load scatter offsets up front, batched within item
            # class so the ds bound is per-class; all scatters on DVE
            # (Pool fails codegen on dynamic-offset tensor_tensor)
            vals_all = []
            n_on_v = nitems
            kbase = 0
            for cw, cnt_cls in zip(ITEM_CLASSES, class_counts):
                done = 0
                while done < cnt_cls:
                    cnt = min(SCATTER_BATCH, cnt_cls - done)
                    if kbase < n_on_v:
                        cnt = min(cnt, n_on_v - kbase)
                    eng = nc.vector if kbase < n_on_v else nc.gpsimd
                    _, vals = nc.values_load_multi_w_load_instructions(
                        off[0:1, kbase:kbase + cnt],
                        engines=[eng.engine],
                        min_val=0,
                        max_val=G - cw,
                        skip_runtime_bounds_check=True,
                    )
                    vals_all.extend(vals)
                    done += cnt
                    kbase += cnt
            assert kbase == nitems

            next_item = [0]

            def emit_ready_scatters(upto_pos):
                k = next_item[0]
                while (k < nitems
                       and geom[k][1] + geom[k][0] <= upto_pos):
                    cw, ppos = geom[k]
                    s = ppos // SUPER
                    u = ppos - s * SUPER
                    src = vint4[:, s, :, u:u + cw]
                    dst = acc3[:, :, bass.ds(vals_all[k], cw)]
                    eng = nc.vector if k < n_on_v else nc.gpsimd
                    eng.tensor_tensor(dst, dst, src, op=OP.min)
                    k += 1
                next_item[0] = k

            for s in range(nsuper):
                pal = psump.tile([128, SUPER], f32, tag="ps")
                pa = psump.tile([128, SUPER], f32, tag="ps")
                pb = psump.tile([128, SUPER], f32, tag="ps")
                for h in (0, 1):
                    ho = slice(512 * h, 512 * (h + 1))
                    hi = slice(SUPER * s + 512 * h, SUPER * s + 512 * (h + 1))
                    nc.tensor.matmul(pal[:, ho], xt[0:4, :], rt[0:4, hi])
                    nc.tensor.matmul(pa[:, ho], xt[32:36, :], rt[32:36, hi])
                    nc.tensor.matmul(pb[:, ho], xt[64:68, :], rt[64:68, hi])

                m = workp.tile([128, SUPER], bf16, tag="m")
                o = workp.tile([128, SUPER], bf16, tag="o")
                so = workp.tile([128, SUPER], bf16, tag="so")
                sb = workp.tile([128, SUPER], bf16, tag="sb")
                d2 = workp.tile([128, SUPER], bf16, tag="d2")
                dd = workp.tile([128, SUPER], bf16, tag="dd")
                nd = workp.tile([128, SUPER], bf16, tag="nd")

                # o = max(a-L, -a, 0) = max(pal, relu(-pa)); d2 = o^2 + b^2
                # (PSUM readable by ACT/DVE only, max one PSUM input per op)
                nc.scalar.activation(m[:], pa[:], AF.Relu, scale=-1.0)
                nc.vector.scalar_tensor_tensor(
                    o[:], pal[:], 0.0, m[:], op0=OP.bypass, op1=OP.max)
                nc.vector.tensor_tensor(so[:], o[:], o[:], op=OP.mult)
                nc.scalar.activation(sb[:], pb[:], AF.Square)
                nc.vector.tensor_tensor(d2[:], so[:], sb[:], op=OP.add)
                nc.scalar.activation(dd[:], d2[:], AF.Sqrt)
                nc.vector.tensor_scalar_add(nd[:], dd[:], -1.0)
                for c in range(3):
                    csl = slice(3 * SUPER * s + c * SUPER,
                                3 * SUPER * s + (c + 1) * SUPER)
                    eng = nc.gpsimd if c == 2 else nc.vector
                    eng.tensor_tensor(vint[:, csl], nd[:],
                                      colb[:, csl], op=OP.mult)

                emit_ready_scatters(SUPER * (s + 1))

            # slot merges: pairwise tree, then combined into acc piecewise
            sbase = (packw - D0 * G) // SUPER

            def slot3(d):
                return vint4[:, sbase + d, :, :]

            t01 = workp.tile([128, 3 * G], bf16, tag="mrg")
            t01_3 = t01[:].rearrange("p (c g) -> p c g", c=3)
            nc.vector.tensor_tensor(t01_3[:, :, :], slot3(0), slot3(1),
                                    op=OP.min)
            t23 = workp.tile([128, 3 * G], bf16, tag="mrg")
            t23_3 = t23[:].rearrange("p (c g) -> p c g", c=3)
            nc.vector.tensor_tensor(t23_3[:, :, :], slot3(2), slot3(3),
                                    op=OP.min)
            nc.vector.tensor_tensor(t01[:], t01[:], t23[:], op=OP.min)

            outst = constp.tile([128, 3 * G], f32)
            NPIECE = 4
            for piece in range(NPIECE):
                slp = slice(piece * 3 * G // NPIECE,
                            (piece + 1) * 3 * G // NPIECE)
                nc.vector.tensor_tensor(acc[:, slp], acc[:, slp],
                                        t01[:, slp], op=OP.min)
                nc.scalar.activation(outst[:, slp], acc[:, slp],
                                     AF.Relu, scale=-1.0)
                nc.sync.dma_start(out_d[:, slp], outst[:, slp])

    nc.compile()
    return nc


# ---------------------------------------------------------------- entry

def _prepare(strokes, thicknesses, colors):
    wins_by_block, t, col = _build_blocks(strokes, thicknesses, colors)
    pairs = _pair_blocks(wins_by_block)
    packed = []
    counts = []
    for c in range(N_CORES):
        wins = [w for b in pairs[c] for w in wins_by_block[b]]
        slots, runs = _sweep_pack(wins)
        items = _runs_to_items(runs)
        packed.append((wins, slots, items))
        counts.append([sum(1 for it in items if it[2] == cw)
                       for cw in ITEM_CLASSES])
    class_counts = tuple(max(c[k] for c in counts)
                         for k in range(len(ITEM_CLASSES)))
    in_maps = []
    packw = None
    for c in range(N_CORES):
        wins, slots, items = packed[c]
        tabs, pw = _build_tables(wins, slots, items, class_counts, t, col,
                                 pairs[c])
        assert packw is None or packw == pw
        packw = pw
        in_maps.append(tabs)
    return pairs, in_maps, class_counts, packw


def kernel(strokes, thicknesses, colors):
    _install_ntff_hook()
    from concourse.bass_utils import run_bass_kernel_spmd

    strokes = np.asarray(strokes)
    thicknesses = np.asarray(thicknesses)
    colors = np.asarray(colors)

    pairs, in_maps, class_counts, packw = _prepare(
        strokes, thicknesses, colors)
    key = (class_counts, packw)
    if key not in _PROG_CACHE:
        _PROG_CACHE[key] = _build_program(class_counts, packw)
    nc = _PROG_CACHE[key]

    res = run_bass_kernel_spmd(nc, in_maps, list(range(N_CORES)))

    out = np.zeros((3, G, G), np.float32)
    for c in range(N_CORES):
        o = res.results[c]["out"]                  # (128, 3*G) planar
        for half, b in enumerate(pairs[c]):
            rows = o[half * BH:(half + 1) * BH]    # (64, 3*G)
            for ch in range(3):
                out[ch, BH * b:BH * (b + 1), :] = \
                    rows[:, ch * G:(ch + 1) * G]
    return out


if __name__ == "__main__":
    rng = np.random.default_rng(0)
    s = rng.random((N, 2, 4), np.float32)
    th = rng.random((N, 1), np.float32)
    co = rng.random((N, 3), np.float32)
    g = kernel(s, th, co)
    print("out", g.shape, g.dtype, g.min(), g.max())


# revision 18
# speedup vs baseline: 1.2239x; 1.0763x over previous
"""Bezier stroke renderer on 8 Trainium2 NeuronCores (Bass/Tile SPMD kernel).

Reference semantics: 32 cubic-Bezier strokes, each sampled into a 16-segment
polyline, rasterized onto a 1024x1024 canvas: per pixel and segment,
darkness = clip((2t - dist_to_segment)/(2t), 0, 1), max over segments within a
stroke, then grid = max(grid, darkness * color) over strokes (3 channels).

Strategy (v3):
  - Canvas split into 16 blocks of 64 rows; block pairs assigned to cores by
    local-search balancing; a core's tile is [128 partitions x 1024 cols].
  - Per-column sweep packing: at each canvas column the (<= D0) active
    windows occupy canvas-aligned full-G slots (windows fragment freely at
    column granularity, so slot overflow is exactly max(0, depth-D0));
    excess runs become width-classed items (16/32/64/128 cols), composited
    with dynamic-offset min ops (register-loaded offsets, SPMD-identical
    instruction stream).
  - Distance math in the segment tangent frame, prescaled by 1/(2t):
    3 matmul families (a-L, a, b) with 2-way fp16 coefficient splits (K=4)
    into f32 PSUM superchunks of 1024 cols (2 banks per family).
  - Post-PSUM chain in bf16 (tolerance 2e-2 allows it):
      m = max(a-L, -a)        [gpsimd scalar_tensor_tensor from PSUM]
      o = max(m, 0)           [DVE tensor_scalar, 4x mode]
      so = o*o                [DVE tensor_tensor, 2x]
      sb = b*b                [ACT Square from PSUM]
      d2 = so + sb            [DVE tensor_tensor]
      dd = sqrt(d2)           [ACT]
      nd = dd - 1             [DVE tensor_scalar]
      v_c = nd * col_c        [DVE tensor_tensor vs a DMA-broadcast color
                               plane; planar per-superchunk layout]
  - Composite: acc[128, 3*G] bf16 (3 planes), min-composite: items scatter
    with 3-plane strided APs at dynamic offsets (split DVE/gpsimd), slots
    merge with static 3-plane min ops; tail = piecewise ACT relu(-x) -> f32
    + DMA out.
"""

import sys
import types
import contextlib
import ctypes

sys.path.insert(0, "/opt/trn_rl_repo")

import numpy as np
import ml_dtypes

G = 1024
P = 16
N = 32
N_CORES = 8
BH = 64             # block height (rows)
NB = G // BH        # 16 blocks
D0 = 4              # canvas-aligned slots
SUPER = 1024        # superchunk columns (2 PSUM banks per family)
ITEM_CLASSES = (16, 32, 64, 128)
SCATTER_BATCH = 8

_bf16 = ml_dtypes.bfloat16
_PROG_CACHE = {}
_HOOK_INSTALLED = False


def _install_ntff_hook():
    """Register the NTFF profile hook so run_bass_kernel_spmd(trace=True)
    can measure HW exec time."""
    global _HOOK_INSTALLED
    if _HOOK_INSTALLED:
        return
    _HOOK_INSTALLED = True
    try:
        import antenv
        mod = types.ModuleType("antenv.axon_hooks")
        holder = [None]
        mod.set_axon_ntff_profile_hook = lambda h: holder.__setitem__(0, h)
        mod.get_axon_ntff_profile_hook = lambda: holder[0]
        sys.modules["antenv.axon_hooks"] = mod
        antenv.axon_hooks = mod

        lib = ctypes.CDLL("/opt/axon/libaxon_pjrt.so")
        if not hasattr(lib, "axon_start_nrt_profile"):
            return
        lib.axon_start_nrt_profile.argtypes = [
            ctypes.POINTER(ctypes.c_int64),
            ctypes.c_size_t,
        ]
        lib.axon_start_nrt_profile.restype = ctypes.c_int64
        lib.axon_stop_nrt_profile.argtypes = [ctypes.c_char_p]
        lib.axon_stop_nrt_profile.restype = ctypes.c_int64

        @contextlib.contextmanager
        def _hook(output_dir, device_ids):
            import jax
            jax.devices()
            if device_ids:
                ids = (ctypes.c_int64 * len(device_ids))(*device_ids)
                rc = lib.axon_start_nrt_profile(ids, len(device_ids))
            else:
                rc = lib.axon_start_nrt_profile(None, 0)
            if rc != 0:
                raise RuntimeError(f"axon_start_nrt_profile rc={rc}")
            try:
                yield
            finally:
                n = lib.axon_stop_nrt_profile(str(output_dir).encode())
                print(f"profile: {n} file(s) written to {output_dir}",
                      file=sys.stderr)

        mod.set_axon_ntff_profile_hook(_hook)
    except Exception:
        pass


# ---------------------------------------------------------------- host side

def _bezier_weights_f32(p):
    t = np.arange(p, dtype=np.float64)
    w1 = (p - t) ** 3 / p ** 3
    w2 = 3 * (p - t) ** 2 * t / p ** 3
    w3 = 3 * (p - t) * t ** 2 / p ** 3
    w4 = t ** 3 / p ** 3
    return np.stack([w1, w2, w3, w4]).astype(np.float32)


def _polylines(strokes):
    W = _bezier_weights_f32(P)
    s = strokes.astype(np.float32)
    pts, derivs = s[:, :, :2], s[:, :, 2:]
    before = pts - derivs
    after = pts + derivs
    p1, p2, p3, p4 = pts[:, :-1], after[:, :-1], before[:, 1:], pts[:, 1:]
    cp = np.stack([p1, p2, p3, p4], axis=3)
    sp = np.einsum("nsdk,kp->nspd", cp, W).astype(np.float32)
    sp = sp.reshape(s.shape[0], -1, 2)
    poly = np.concatenate([sp, pts[:, -1:, :]], axis=1).astype(np.float32)
    return poly * np.float32(G)


def _band_clip(v, w, pad, x0, x1):
    lo_x, hi_x = x0 - pad, x1 + pad
    dx = w[0] - v[0]
    if abs(dx) < 1e-12:
        if v[0] < lo_x or v[0] > hi_x:
            return None
        s0, s1 = 0.0, 1.0
    else:
        sa = (lo_x - v[0]) / dx
        sb = (hi_x - v[0]) / dx
        s0 = max(0.0, min(sa, sb))
        s1 = min(1.0, max(sa, sb))
        if s0 > s1:
            return None
    ya = v[1] + s0 * (w[1] - v[1])
    yb = v[1] + s1 * (w[1] - v[1])
    c0 = max(0.0, min(ya, yb) - pad)
    c1 = min(G - 1.0, max(ya, yb) + pad)
    if c1 < c0:
        return None
    return int(np.floor(c0)), int(np.ceil(c1))


def _build_blocks(strokes, thicknesses, colors):
    poly = _polylines(strokes).astype(np.float64)
    t = np.maximum(thicknesses.astype(np.float32) * np.float32(2.0)
                   + np.float32(0.5), np.float32(0.5))[:, 0]
    col = np.clip(colors.astype(np.float32), 0.0, 1.0)
    pad = 2.0 * t.astype(np.float64) + 1.0
    wins_by_block = [[] for _ in range(NB)]
    for n in range(N):
        for i in range(P):
            v = poly[n, i]
            w = poly[n, i + 1]
            for b in range(NB):
                clip = _band_clip(v, w, pad[n], BH * b, BH * b + BH - 1)
                if clip is None:
                    continue
                c0, c1 = clip
                wins_by_block[b].append((n, v, w, c0, c1))
    return wins_by_block, t, col


def _depth_profile(wins):
    d = np.zeros(G, np.int64)
    for (_, _, _, c0, c1) in wins:
        d[c0:c1 + 1] += 1
    return d


def _pair_blocks(wins_by_block):
    """Pair blocks two-per-core, local-search minimizing per-core excess."""
    profs = [_depth_profile(wins_by_block[b]) for b in range(NB)]

    def cost(b1, b2):
        d = profs[b1] + profs[b2]
        return np.maximum(d - D0, 0).sum(), d.sum()

    order = sorted(range(NB),
                   key=lambda b: -np.maximum(profs[b] - D0, 0).sum())
    pairs = [[order[i], order[NB - 1 - i]] for i in range(NB // 2)]

    def metric(ps):
        cs = [cost(p[0], p[1]) for p in ps]
        return (max(c[0] for c in cs), sum(c[0] for c in cs),
                max(c[1] for c in cs))

    best = metric(pairs)
    improved = True
    while improved:
        improved = False
        for i in range(len(pairs)):
            for j in range(i + 1, len(pairs)):
                for a in range(2):
                    for b in range(2):
                        pairs[i][a], pairs[j][b] = pairs[j][b], pairs[i][a]
                        m = metric(pairs)
                        if m < best:
                            best = m
                            improved = True
                        else:
                            pairs[i][a], pairs[j][b] = pairs[j][b], pairs[i][a]
    return [sorted(p) for p in pairs]


def _sweep_pack(wins):
    """Per-column sweep: active windows (arrival order) -> slot levels;
    excess -> maximal runs. Returns (slots (D0,G) win-index or -1, runs)."""
    evs = sorted(range(len(wins)), key=lambda i: wins[i][3])
    slots = np.full((D0, G), -1, np.int64)
    active = []
    ei = 0
    run_open = {}
    runs = []
    for y in range(G):
        while ei < len(evs) and wins[evs[ei]][3] == y:
            active.append((evs[ei], wins[evs[ei]][4]))
            ei += 1
        active = [(i, c1) for (i, c1) in active if c1 >= y]
        excess_now = set()
        for lvl, (i, c1) in enumerate(active):
            if lvl < D0:
                slots[lvl, y] = i
            else:
                excess_now.add(i)
        for i in list(run_open):
            if i not in excess_now:
                runs.append((i, run_open.pop(i), y - 1))
        for i in excess_now:
            if i not in run_open:
                run_open[i] = y
    for i, c0 in run_open.items():
        runs.append((i, c0, G - 1))
    return slots, runs


def _runs_to_items(runs):
    items = []
    for (i, c0, c1) in runs:
        w = c1 - c0 + 1
        start = c0
        while w > 0:
            take = min(ITEM_CLASSES[-1], w)
            cls = min(c for c in ITEM_CLASSES if c >= take)
            items.append((i, max(0, min(start, G - cls)), cls))
            start += take
            w -= take
    return items


def _split2(v):
    h = v.astype(np.float16)
    l = (v - h.astype(np.float64)).astype(np.float16)
    return h, l


def _item_geometry(class_counts):
    """Packed (class_w, packed_pos) per item, never crossing a SUPER
    boundary; returns (geom, item_region_w) with item_region_w a multiple
    of nothing in particular (caller pads)."""
    geom = []
    pos = 0
    for cw, cnt in zip(ITEM_CLASSES, class_counts):
        for _ in range(cnt):
            if pos % SUPER + cw > SUPER:
                pos = (pos // SUPER + 1) * SUPER
            geom.append((cw, pos))
            pos += cw
    return geom, pos


def _layout(class_counts):
    geom, item_w = _item_geometry(class_counts)
    packw = -(-(item_w + D0 * G) // SUPER) * SUPER
    return geom, packw


def _build_tables(wins, slots, items, class_counts, t, col, blocks):
    geom, packw = _layout(class_counts)

    widx = np.full(packw, -1, np.int64)
    ycol = np.zeros(packw, np.float64)
    offs = []
    by_class = {cw: [] for cw in ITEM_CLASSES}
    for (i, c0, cls) in items:
        by_class[cls].append((i, c0))
    ki = 0
    for cw, cnt in zip(ITEM_CLASSES, class_counts):
        lst = by_class[cw]
        assert len(lst) <= cnt
        for k in range(cnt):
            cwg, pos = geom[ki]
            assert cwg == cw
            if k < len(lst):
                i, c0 = lst[k]
                widx[pos:pos + cw] = i
                ycol[pos:pos + cw] = c0 + np.arange(cw)
                offs.append(c0)
            else:
                offs.append(0)
            ki += 1
    pos = packw - D0 * G
    for d in range(D0):
        widx[pos:pos + G] = slots[d]
        ycol[pos:pos + G] = np.arange(G)
        pos += G
    assert pos == packw

    nw = len(wins)
    vx = np.array([w[1][0] for w in wins] + [0.0])
    vy = np.array([w[1][1] for w in wins] + [0.0])
    wx = np.array([w[2][0] for w in wins] + [0.0])
    wy = np.array([w[2][1] for w in wins] + [0.0])
    tn = np.array([t[w[0]] for w in wins] + [1.0], np.float64)
    cn = np.array([col[w[0]] for w in wins] + [[0.0, 0.0, 0.0]], np.float64)

    wi = np.where(widx < 0, nw, widx)
    dead = widx < 0
    i2t = 1.0 / (2.0 * tn[wi])
    dx = wx[wi] - vx[wi]
    dy = wy[wi] - vy[wi]
    L = np.hypot(dx, dy)
    safe = L > 1e-9
    taux = np.where(safe, dx / np.where(safe, L, 1.0), 1.0)
    tauy = np.where(safe, dy / np.where(safe, L, 1.0), 0.0)
    Leff = np.where(safe, L, 0.0)
    nux, nuy = -tauy, taux
    av = vx[wi] * taux + vy[wi] * tauy
    bv = vx[wi] * nux + vy[wi] * nuy
    a1 = taux * i2t
    a2 = (ycol * tauy - av) * i2t
    b1 = nux * i2t
    b2 = (ycol * nuy - bv) * i2t
    ll = Leff * i2t
    for arr in (a1, a2, b1, b2, ll):
        arr[dead] = 0.0
    colp = cn[wi].T.copy()
    colp[:, dead] = 0.0

    rt = np.zeros((12, packw), np.float16)
    for f, (const, xc) in enumerate(((a2 - ll, a1), (a2, a1), (b2, b1))):
        ch, cl = _split2(const)
        xh, xl = _split2(xc)
        rt[4 * f + 0] = ch
        rt[4 * f + 1] = cl
        rt[4 * f + 2] = xh
        rt[4 * f + 3] = xl

    # colb: planar per superchunk [c0-plane | c1-plane | c2-plane] x nsuper,
    # broadcast to 128 partitions
    nsuper = packw // SUPER
    colrow = np.zeros(3 * packw, np.float64)
    for s in range(nsuper):
        for c in range(3):
            colrow[3 * SUPER * s + c * SUPER:
                   3 * SUPER * s + (c + 1) * SUPER] = \
                colp[c, SUPER * s:SUPER * (s + 1)]
    colb = np.broadcast_to(colrow.astype(_bf16), (128, 3 * packw))
    colb = np.ascontiguousarray(colb)

    xs = np.zeros(128, np.float64)
    for half, b in enumerate(blocks):
        xs[half * BH:(half + 1) * BH] = BH * b + np.arange(BH)
    xt = np.zeros((68, 128), np.float16)
    for base in (0, 32, 64):
        xt[base + 0:base + 2] = 1.0
        xt[base + 2:base + 4] = xs.astype(np.float16)  # exact (< 2048)

    off = np.array(offs or [0], np.int32).reshape(1, -1)
    return dict(xt=xt, rt=rt, colb=colb, off=off), packw


# ---------------------------------------------------------------- bass side

def _build_program(class_counts, packw):
    import concourse.bacc as bacc
    import concourse.mybir as mybir
    import concourse.bass as bass
    from concourse import tile

    f32 = mybir.dt.float32
    f16 = mybir.dt.float16
    bf16 = mybir.dt.bfloat16
    i32 = mybir.dt.int32
    AF = mybir.ActivationFunctionType
    OP = mybir.AluOpType

    nitems = sum(class_counts)
    geom, packw2 = _layout(class_counts)
    assert packw2 == packw
    nsuper = packw // SUPER

    nc = bacc.Bacc("TRN2", target_bir_lowering=False, debug=False,
                   num_devices=N_CORES)
    xt_d = nc.dram_tensor("xt", [68, 128], f16, kind="ExternalInput").ap()
    rt_d = nc.dram_tensor("rt", [12, packw], f16, kind="ExternalInput").ap()
    colb_d = nc.dram_tensor("colb", [128, 3 * packw], bf16,
                            kind="ExternalInput").ap()
    off_d = nc.dram_tensor("off", [1, max(nitems, 1)], i32,
                           kind="ExternalInput").ap()
    out_d = nc.dram_tensor("out", [128, 3 * G], f32, kind="ExternalOutput").ap()

    with tile.TileContext(nc) as tc:
        with (
            tc.tile_pool(name="const", bufs=1) as constp,
            tc.tile_pool(name="work", bufs=3) as workp,
            tc.tile_pool(name="psum", bufs=4, space="PSUM") as psump,
        ):
            xt = constp.tile([68, 128], f16)
            nc.sync.dma_start(xt[:], xt_d[:])
            # warmup matmul to kick the PE out of its low-power pstate early
            # (read back by a cheap DVE op so the psum ring buffer recycles)
            warm = psump.tile([128, SUPER], f32, tag="ps")
            nc.tensor.matmul(warm[:, 0:128], xt[0:4, :], xt[0:4, :])
            wdump = workp.tile([128, 128], f32, tag="wd")
            nc.vector.tensor_scalar_mul(wdump[:], warm[:, 0:128], 0.0)
            rt = constp.tile([68, packw], f16)
            half_w = (nsuper // 2) * SUPER
            for csl in (slice(0, half_w), slice(half_w, packw)):
                nc.sync.dma_start(rt[0:4, csl], rt_d[0:4, csl])
                nc.sync.dma_start(rt[32:36, csl], rt_d[4:8, csl])
                nc.sync.dma_start(rt[64:68, csl], rt_d[8:12, csl])
            off = constp.tile([1, max(nitems, 1)], i32)
            nc.sync.dma_start(off[:], off_d[:])
            colb = constp.tile([128, 3 * packw], bf16)
            for s in range(nsuper):
                sl3 = slice(3 * SUPER * s, 3 * SUPER * (s + 1))
                nc.sync.dma_start(colb[:, sl3], colb_d[:, sl3])

            vint = constp.tile([128, 3 * packw], bf16)
            acc = constp.tile([128, 3 * G], bf16)
            nc.gpsimd.memset(acc[:], 0.0)

            acc3 = acc[:].rearrange("p (c g) -> p c g", c=3)
            vint4 = vint[:].rearrange("p (s c g) -> p s c g", s=nsuper, c=3)

            # register-load scatter offsets up front, batched within item
            # class so the ds bound is per-class; all scatters on DVE
            # (Pool fails codegen on dynamic-offset tensor_tensor)
            vals_all = []
            n_on_v = nitems
            kbase = 0
            for cw, cnt_cls in zip(ITEM_CLASSES, class_counts):
                done = 0
                while done < cnt_cls:
                    cnt = min(SCATTER_BATCH, cnt_cls - done)
                    if kbase < n_on_v:
                        cnt = min(cnt, n_on_v - kbase)
                    eng = nc.vector if kbase < n_on_v else nc.gpsimd
                    _, vals = nc.values_load_multi_w_load_instructions(
                        off[0:1, kbase:kbase + cnt],
                        engines=[eng.engine],
                        min_val=0,
                        max_val=G - cw,
                        skip_runtime_bounds_check=True,
                    )
                    vals_all.extend(vals)
                    done += cnt
                    kbase += cnt
            assert kbase == nitems

            sbase = (packw - D0 * G) // SUPER

            def slot3(d):
                return vint4[:, sbase + d, :, :]

            taccs = constp.tile([128, 3 * G], bf16)
            taccs3 = taccs[:].rearrange("p (c g) -> p c g", c=3)

            next_item = [0]

            def emit_ready_scatters(upto_pos):
                k = next_item[0]
                while (k < nitems
                       and geom[k][1] + geom[k][0] <= upto_pos):
                    cw, ppos = geom[k]
                    s = ppos // SUPER
                    u = ppos - s * SUPER
                    src = vint4[:, s, :, u:u + cw]
                    dst = acc3[:, :, bass.ds(vals_all[k], cw)]
                    eng = nc.vector if k < n_on_v else nc.gpsimd
                    eng.tensor_tensor(dst, dst, src, op=OP.min)
                    k += 1
                next_item[0] = k

            for s in range(nsuper):
                pal = psump.tile([128, SUPER], f32, tag="ps")
                pa = psump.tile([128, SUPER], f32, tag="ps")
                pb = psump.tile([128, SUPER], f32, tag="ps")
                for h in (0, 1):
                    ho = slice(512 * h, 512 * (h + 1))
                    hi = slice(SUPER * s + 512 * h, SUPER * s + 512 * (h + 1))
                    nc.tensor.matmul(pal[:, ho], xt[0:4, :], rt[0:4, hi])
                    nc.tensor.matmul(pa[:, ho], xt[32:36, :], rt[32:36, hi])
                    nc.tensor.matmul(pb[:, ho], xt[64:68, :], rt[64:68, hi])

                m = workp.tile([128, SUPER], bf16, tag="m")
                o = workp.tile([128, SUPER], bf16, tag="o")
                so = workp.tile([128, SUPER], bf16, tag="so")
                sb = workp.tile([128, SUPER], bf16, tag="sb")
                d2 = workp.tile([128, SUPER], bf16, tag="d2")
                dd = workp.tile([128, SUPER], bf16, tag="dd")
                nd = workp.tile([128, SUPER], bf16, tag="nd")

                # o = max(a-L, -a, 0) = max(pal, relu(-pa)); d2 = o^2 + b^2
                # (PSUM readable by ACT/DVE only, max one PSUM input per op)
                nc.scalar.activation(m[:], pa[:], AF.Relu, scale=-1.0)
                nc.vector.scalar_tensor_tensor(
                    o[:], pal[:], 0.0, m[:], op0=OP.bypass, op1=OP.max)
                nc.scalar.activation(so[:], o[:], AF.Square)
                nc.scalar.activation(sb[:], pb[:], AF.Square)
                nc.vector.tensor_tensor(d2[:], so[:], sb[:], op=OP.add)
                nc.scalar.activation(dd[:], d2[:], AF.Sqrt)
                nc.vector.tensor_scalar_add(nd[:], dd[:], -1.0)
                has_items = next_item[0] < nitems and \
                    geom[next_item[0]][1] < SUPER * (s + 1)
                for c in range(3):
                    csl = slice(3 * SUPER * s + c * SUPER,
                                3 * SUPER * s + (c + 1) * SUPER)
                    # scatters wait on all three planes: keep item
                    # superchunks all-DVE (gpsimd's mult is ~2.5us)
                    eng = nc.gpsimd if (c == 2 and not has_items) \
                        else nc.vector
                    eng.tensor_tensor(vint[:, csl], nd[:],
                                      colb[:, csl], op=OP.mult)

                emit_ready_scatters(SUPER * (s + 1))

                # rolling slot merges (pipelined with later superchunks)
                d = s - sbase
                if d == 1:
                    nc.vector.tensor_tensor(taccs3[:, :, :], slot3(0),
                                            slot3(1), op=OP.min)
                elif d > 1:
                    nc.vector.tensor_tensor(taccs3[:, :, :], taccs3[:, :, :],
                                            slot3(d), op=OP.min)

            # combine scatter acc with rolled slot merge, negate, store
            outst = constp.tile([128, 3 * G], f32)
            NPIECE = 4
            for piece in range(NPIECE):
                slp = slice(piece * 3 * G // NPIECE,
                            (piece + 1) * 3 * G // NPIECE)
                nc.vector.tensor_tensor(acc[:, slp], acc[:, slp],
                                        taccs[:, slp], op=OP.min)
                nc.scalar.activation(outst[:, slp], acc[:, slp],
                                     AF.Relu, scale=-1.0)
                nc.sync.dma_start(out_d[:, slp], outst[:, slp])

    nc.compile()
    return nc


# ---------------------------------------------------------------- entry

def _prepare(strokes, thicknesses, colors):
    wins_by_block, t, col = _build_blocks(strokes, thicknesses, colors)
    pairs = _pair_blocks(wins_by_block)
    packed = []
    counts = []
    for c in range(N_CORES):
        wins = [w for b in pairs[c] for w in wins_by_block[b]]
        slots, runs = _sweep_pack(wins)
        items = _runs_to_items(runs)
        packed.append((wins, slots, items))
        counts.append([sum(1 for it in items if it[2] == cw)
                       for cw in ITEM_CLASSES])
    class_counts = tuple(max(c[k] for c in counts)
                         for k in range(len(ITEM_CLASSES)))
    in_maps = []
    packw = None
    for c in range(N_CORES):
        wins, slots, items = packed[c]
        tabs, pw = _build_tables(wins, slots, items, class_counts, t, col,
                                 pairs[c])
        assert packw is None or packw == pw
        packw = pw
        in_maps.append(tabs)
    return pairs, in_maps, class_counts, packw


def kernel(strokes, thicknesses, colors):
    _install_ntff_hook()
    from concourse.bass_utils import run_bass_kernel_spmd

    strokes = np.asarray(strokes)
    thicknesses = np.asarray(thicknesses)
    colors = np.asarray(colors)

    pairs, in_maps, class_counts, packw = _prepare(
        strokes, thicknesses, colors)
    key = (class_counts, packw)
    if key not in _PROG_CACHE:
        _PROG_CACHE[key] = _build_program(class_counts, packw)
    nc = _PROG_CACHE[key]

    res = run_bass_kernel_spmd(nc, in_maps, list(range(N_CORES)))

    out = np.zeros((3, G, G), np.float32)
    for c in range(N_CORES):
        o = res.results[c]["out"]                  # (128, 3*G) planar
        for half, b in enumerate(pairs[c]):
            rows = o[half * BH:(half + 1) * BH]    # (64, 3*G)
            for ch in range(3):
                out[ch, BH * b:BH * (b + 1), :] = \
                    rows[:, ch * G:(ch + 1) * G]
    return out


if __name__ == "__main__":
    rng = np.random.default_rng(0)
    s = rng.random((N, 2, 4), np.float32)
    th = rng.random((N, 1), np.float32)
    co = rng.random((N, 3), np.float32)
    g = kernel(s, th, co)
    print("out", g.shape, g.dtype, g.min(), g.max())


# revision 27
# speedup vs baseline: 1.2364x; 1.0103x over previous
"""Bezier stroke renderer on 8 Trainium2 NeuronCores (Bass/Tile SPMD kernel).

Reference semantics: 32 cubic-Bezier strokes, each sampled into a 16-segment
polyline, rasterized onto a 1024x1024 canvas: per pixel and segment,
darkness = clip((2t - dist_to_segment)/(2t), 0, 1), max over segments within a
stroke, then grid = max(grid, darkness * color) over strokes (3 channels).

Strategy (v3):
  - Canvas split into 16 blocks of 64 rows; block pairs assigned to cores by
    local-search balancing; a core's tile is [128 partitions x 1024 cols].
  - Per-column sweep packing: at each canvas column the (<= D0) active
    windows occupy canvas-aligned full-G slots (windows fragment freely at
    column granularity, so slot overflow is exactly max(0, depth-D0));
    excess runs become width-classed items (16/32/64/128 cols), composited
    with dynamic-offset min ops (register-loaded offsets, SPMD-identical
    instruction stream).
  - Distance math in the segment tangent frame, prescaled by 1/(2t):
    3 matmul families (a-L, a, b) with 2-way fp16 coefficient splits (K=4)
    into f32 PSUM superchunks of 1024 cols (2 banks per family).
  - Post-PSUM chain in bf16 (tolerance 2e-2 allows it):
      m = max(a-L, -a)        [gpsimd scalar_tensor_tensor from PSUM]
      o = max(m, 0)           [DVE tensor_scalar, 4x mode]
      so = o*o                [DVE tensor_tensor, 2x]
      sb = b*b                [ACT Square from PSUM]
      d2 = so + sb            [DVE tensor_tensor]
      dd = sqrt(d2)           [ACT]
      nd = dd - 1             [DVE tensor_scalar]
      v_c = nd * col_c        [DVE tensor_tensor vs a DMA-broadcast color
                               plane; planar per-superchunk layout]
  - Composite: acc[128, 3*G] bf16 (3 planes), min-composite: items scatter
    with 3-plane strided APs at dynamic offsets (split DVE/gpsimd), slots
    merge with static 3-plane min ops; tail = piecewise ACT relu(-x) -> f32
    + DMA out.
"""

import sys
import types
import contextlib
import ctypes

sys.path.insert(0, "/opt/trn_rl_repo")

import numpy as np
import ml_dtypes

G = 1024
P = 16
N = 32
N_CORES = 8
BH = 64             # block height (rows)
NB = G // BH        # 16 blocks
D0 = 6              # canvas-aligned slots
SUPER = 1024        # superchunk columns (2 PSUM banks per family)
ITEM_CLASSES = (32, 64, 96, 128)
STRIP_GAP = 48      # close a strip when the next excess column is further
SCATTER_BATCH = 8

_bf16 = ml_dtypes.bfloat16
_PROG_CACHE = {}
_HOOK_INSTALLED = False


def _install_ntff_hook():
    """Register the NTFF profile hook so run_bass_kernel_spmd(trace=True)
    can measure HW exec time."""
    global _HOOK_INSTALLED
    if _HOOK_INSTALLED:
        return
    _HOOK_INSTALLED = True
    try:
        import antenv
        mod = types.ModuleType("antenv.axon_hooks")
        holder = [None]
        mod.set_axon_ntff_profile_hook = lambda h: holder.__setitem__(0, h)
        mod.get_axon_ntff_profile_hook = lambda: holder[0]
        sys.modules["antenv.axon_hooks"] = mod
        antenv.axon_hooks = mod

        lib = ctypes.CDLL("/opt/axon/libaxon_pjrt.so")
        if not hasattr(lib, "axon_start_nrt_profile"):
            return
        lib.axon_start_nrt_profile.argtypes = [
            ctypes.POINTER(ctypes.c_int64),
            ctypes.c_size_t,
        ]
        lib.axon_start_nrt_profile.restype = ctypes.c_int64
        lib.axon_stop_nrt_profile.argtypes = [ctypes.c_char_p]
        lib.axon_stop_nrt_profile.restype = ctypes.c_int64

        @contextlib.contextmanager
        def _hook(output_dir, device_ids):
            import jax
            jax.devices()
            if device_ids:
                ids = (ctypes.c_int64 * len(device_ids))(*device_ids)
                rc = lib.axon_start_nrt_profile(ids, len(device_ids))
            else:
                rc = lib.axon_start_nrt_profile(None, 0)
            if rc != 0:
                raise RuntimeError(f"axon_start_nrt_profile rc={rc}")
            try:
                yield
            finally:
                n = lib.axon_stop_nrt_profile(str(output_dir).encode())
                print(f"profile: {n} file(s) written to {output_dir}",
                      file=sys.stderr)

        mod.set_axon_ntff_profile_hook(_hook)
    except Exception:
        pass


# ---------------------------------------------------------------- host side

def _bezier_weights_f32(p):
    t = np.arange(p, dtype=np.float64)
    w1 = (p - t) ** 3 / p ** 3
    w2 = 3 * (p - t) ** 2 * t / p ** 3
    w3 = 3 * (p - t) * t ** 2 / p ** 3
    w4 = t ** 3 / p ** 3
    return np.stack([w1, w2, w3, w4]).astype(np.float32)


def _polylines(strokes):
    W = _bezier_weights_f32(P)
    s = strokes.astype(np.float32)
    pts, derivs = s[:, :, :2], s[:, :, 2:]
    before = pts - derivs
    after = pts + derivs
    p1, p2, p3, p4 = pts[:, :-1], after[:, :-1], before[:, 1:], pts[:, 1:]
    cp = np.stack([p1, p2, p3, p4], axis=3)
    sp = np.einsum("nsdk,kp->nspd", cp, W).astype(np.float32)
    sp = sp.reshape(s.shape[0], -1, 2)
    poly = np.concatenate([sp, pts[:, -1:, :]], axis=1).astype(np.float32)
    return poly * np.float32(G)


def _band_clip(v, w, pad, x0, x1):
    lo_x, hi_x = x0 - pad, x1 + pad
    dx = w[0] - v[0]
    if abs(dx) < 1e-12:
        if v[0] < lo_x or v[0] > hi_x:
            return None
        s0, s1 = 0.0, 1.0
    else:
        sa = (lo_x - v[0]) / dx
        sb = (hi_x - v[0]) / dx
        s0 = max(0.0, min(sa, sb))
        s1 = min(1.0, max(sa, sb))
        if s0 > s1:
            return None
    ya = v[1] + s0 * (w[1] - v[1])
    yb = v[1] + s1 * (w[1] - v[1])
    c0 = max(0.0, min(ya, yb) - pad)
    c1 = min(G - 1.0, max(ya, yb) + pad)
    if c1 < c0:
        return None
    return int(np.floor(c0)), int(np.ceil(c1))


def _build_blocks(strokes, thicknesses, colors):
    poly = _polylines(strokes).astype(np.float64)
    t = np.maximum(thicknesses.astype(np.float32) * np.float32(2.0)
                   + np.float32(0.5), np.float32(0.5))[:, 0]
    col = np.clip(colors.astype(np.float32), 0.0, 1.0)
    pad = 2.0 * t.astype(np.float64) + 1.0
    wins_by_block = [[] for _ in range(NB)]
    for n in range(N):
        for i in range(P):
            v = poly[n, i]
            w = poly[n, i + 1]
            for b in range(NB):
                clip = _band_clip(v, w, pad[n], BH * b, BH * b + BH - 1)
                if clip is None:
                    continue
                c0, c1 = clip
                wins_by_block[b].append((n, v, w, c0, c1))
    return wins_by_block, t, col


def _depth_profile(wins):
    d = np.zeros(G, np.int64)
    for (_, _, _, c0, c1) in wins:
        d[c0:c1 + 1] += 1
    return d


def _pair_blocks(wins_by_block):
    """Pair blocks two-per-core, local-search minimizing the true packing
    objective: (packw, total scatter items)."""
    profs = [_depth_profile(wins_by_block[b]) for b in range(NB)]

    def metric(ps):
        counts = []
        for p in ps:
            wins = [w for b in p for w in wins_by_block[b]]
            _, exc = _sweep_pack(wins)
            items = _build_strips(exc)
            counts.append([sum(1 for it in items if it[1] == cw)
                           for cw in ITEM_CLASSES])
        cc = tuple(max(c[k] for c in counts)
                   for k in range(len(ITEM_CLASSES)))
        _, packw = _layout(cc)
        return packw, sum(cc)

    order = sorted(range(NB),
                   key=lambda b: -np.maximum(profs[b] - D0, 0).sum())
    pairs = [[order[i], order[NB - 1 - i]] for i in range(NB // 2)]
    best = metric(pairs)
    for _ in range(6):
        improved = False
        for i in range(len(pairs)):
            for j in range(i + 1, len(pairs)):
                for a in range(2):
                    for b in range(2):
                        pairs[i][a], pairs[j][b] = pairs[j][b], pairs[i][a]
                        m = metric(pairs)
                        if m < best:
                            best = m
                            improved = True
                        else:
                            pairs[i][a], pairs[j][b] = pairs[j][b], pairs[i][a]
        if not improved:
            break
    return [sorted(p) for p in pairs]


def _sweep_pack(wins):
    """Per-column sweep: active windows (arrival order) -> slot levels;
    excess -> per-column lists. Returns (slots (D0,G) win-index or -1,
    excess_cols: list[G] of lists of window indices)."""
    evs = sorted(range(len(wins)), key=lambda i: wins[i][3])
    slots = np.full((D0, G), -1, np.int64)
    active = []
    ei = 0
    excess_cols = [[] for _ in range(G)]
    for y in range(G):
        while ei < len(evs) and wins[evs[ei]][3] == y:
            active.append((evs[ei], wins[evs[ei]][4]))
            ei += 1
        active = [(i, c1) for (i, c1) in active if c1 >= y]
        for lvl, (i, c1) in enumerate(active):
            if lvl < D0:
                slots[lvl, y] = i
            else:
                excess_cols[y].append(i)
    return slots, excess_cols


def _build_strips(excess_cols):
    """Cover excess demand with canvas strips: strip = (c0, class_w,
    colmap {y: win}) holding one excess level over a contiguous span
    (dead gaps inside are neutral)."""
    strips = []
    max_lvl = max((len(e) for e in excess_cols), default=0)
    wmax = ITEM_CLASSES[-1]
    for lvl in range(max_lvl):
        cols = [y for y in range(G) if len(excess_cols[y]) > lvl]
        i = 0
        while i < len(cols):
            start = cols[i]
            last = start
            cover = [cols[i]]
            i += 1
            while i < len(cols) and cols[i] - start < wmax and \
                    cols[i] - last <= STRIP_GAP:
                last = cols[i]
                cover.append(cols[i])
                i += 1
            width = last - start + 1
            cls = min(c for c in ITEM_CLASSES if c >= width)
            c0 = max(0, min(start, G - cls))
            strips.append((c0, cls, {y: excess_cols[y][lvl] for y in cover}))
    return strips


def _split2(v):
    h = v.astype(np.float16)
    l = (v - h.astype(np.float64)).astype(np.float16)
    return h, l


def _item_geometry(class_counts):
    """Packed (class_w, packed_pos) per item, never crossing a SUPER
    boundary; returns (geom, item_region_w) with item_region_w a multiple
    of nothing in particular (caller pads)."""
    geom = []
    pos = 0
    for cw, cnt in zip(ITEM_CLASSES, class_counts):
        for _ in range(cnt):
            if pos % SUPER + cw > SUPER:
                pos = (pos // SUPER + 1) * SUPER
            geom.append((cw, pos))
            pos += cw
    return geom, pos


def _layout(class_counts):
    geom, item_w = _item_geometry(class_counts)
    packw = -(-(item_w + D0 * G) // SUPER) * SUPER
    return geom, packw


def _build_tables(wins, slots, items, class_counts, t, col, blocks):
    geom, packw = _layout(class_counts)

    widx = np.full(packw, -1, np.int64)
    ycol = np.zeros(packw, np.float64)
    offs = []
    by_class = {cw: [] for cw in ITEM_CLASSES}
    for (c0, cls, colmap) in items:
        by_class[cls].append((c0, colmap))
    ki = 0
    for cw, cnt in zip(ITEM_CLASSES, class_counts):
        lst = by_class[cw]
        assert len(lst) <= cnt
        for k in range(cnt):
            cwg, pos = geom[ki]
            assert cwg == cw
            if k < len(lst):
                c0, colmap = lst[k]
                for y, win in colmap.items():
                    widx[pos + (y - c0)] = win
                    ycol[pos + (y - c0)] = y
                offs.append(c0)
            else:
                offs.append(0)
            ki += 1
    pos = packw - D0 * G
    for d in range(D0):
        widx[pos:pos + G] = slots[d]
        ycol[pos:pos + G] = np.arange(G)
        pos += G
    assert pos == packw

    nw = len(wins)
    vx = np.array([w[1][0] for w in wins] + [0.0])
    vy = np.array([w[1][1] for w in wins] + [0.0])
    wx = np.array([w[2][0] for w in wins] + [0.0])
    wy = np.array([w[2][1] for w in wins] + [0.0])
    tn = np.array([t[w[0]] for w in wins] + [1.0], np.float64)
    cn = np.array([col[w[0]] for w in wins] + [[0.0, 0.0, 0.0]], np.float64)

    wi = np.where(widx < 0, nw, widx)
    dead = widx < 0
    i2t = 1.0 / (2.0 * tn[wi])
    dx = wx[wi] - vx[wi]
    dy = wy[wi] - vy[wi]
    L = np.hypot(dx, dy)
    safe = L > 1e-9
    taux = np.where(safe, dx / np.where(safe, L, 1.0), 1.0)
    tauy = np.where(safe, dy / np.where(safe, L, 1.0), 0.0)
    Leff = np.where(safe, L, 0.0)
    nux, nuy = -tauy, taux
    av = vx[wi] * taux + vy[wi] * tauy
    bv = vx[wi] * nux + vy[wi] * nuy
    a1 = taux * i2t
    a2 = (ycol * tauy - av) * i2t
    b1 = nux * i2t
    b2 = (ycol * nuy - bv) * i2t
    ll = Leff * i2t
    for arr in (a1, a2, b1, b2, ll):
        arr[dead] = 0.0
    colp = cn[wi].T.copy()
    colp[:, dead] = 0.0

    rt = np.zeros((12, packw), np.float16)
    for f, (const, xc) in enumerate(((a2 - ll, a1), (a2, a1), (b2, b1))):
        ch, cl = _split2(const)
        xh, xl = _split2(xc)
        rt[4 * f + 0] = ch
        rt[4 * f + 1] = cl
        rt[4 * f + 2] = xh
        rt[4 * f + 3] = xl

    # colb: planar per superchunk [c0-plane | c1-plane | c2-plane] x nsuper,
    # broadcast to 128 partitions
    nsuper = packw // SUPER
    colrow = np.zeros(3 * packw, np.float64)
    for s in range(nsuper):
        for c in range(3):
            colrow[3 * SUPER * s + c * SUPER:
                   3 * SUPER * s + (c + 1) * SUPER] = \
                colp[c, SUPER * s:SUPER * (s + 1)]
    colb = np.broadcast_to(colrow.astype(_bf16), (128, 3 * packw))
    colb = np.ascontiguousarray(colb)

    xs = np.zeros(128, np.float64)
    for half, b in enumerate(blocks):
        xs[half * BH:(half + 1) * BH] = BH * b + np.arange(BH)
    xt = np.zeros((68, 128), np.float16)
    for base in (0, 32, 64):
        xt[base + 0:base + 2] = 1.0
        xt[base + 2:base + 4] = xs.astype(np.float16)  # exact (< 2048)

    off = np.array(offs or [0], np.int32).reshape(1, -1)
    return dict(xt=xt, rt=rt, colb=colb, off=off), packw


# ---------------------------------------------------------------- bass side

def _build_program(class_counts, packw):
    import concourse.bacc as bacc
    import concourse.mybir as mybir
    import concourse.bass as bass
    from concourse import tile

    f32 = mybir.dt.float32
    f16 = mybir.dt.float16
    bf16 = mybir.dt.bfloat16
    i32 = mybir.dt.int32
    AF = mybir.ActivationFunctionType
    OP = mybir.AluOpType

    nitems = sum(class_counts)
    geom, packw2 = _layout(class_counts)
    assert packw2 == packw
    nsuper = packw // SUPER

    nc = bacc.Bacc("TRN2", target_bir_lowering=False, debug=False,
                   num_devices=N_CORES)
    xt_d = nc.dram_tensor("xt", [68, 128], f16, kind="ExternalInput").ap()
    rt_d = nc.dram_tensor("rt", [12, packw], f16, kind="ExternalInput").ap()
    colb_d = nc.dram_tensor("colb", [128, 3 * packw], bf16,
                            kind="ExternalInput").ap()
    off_d = nc.dram_tensor("off", [1, max(nitems, 1)], i32,
                           kind="ExternalInput").ap()
    out_d = nc.dram_tensor("out", [128, 3 * G], f32, kind="ExternalOutput").ap()

    with tile.TileContext(nc) as tc:
        with (
            tc.tile_pool(name="const", bufs=1) as constp,
            tc.tile_pool(name="work", bufs=3) as workp,
            tc.tile_pool(name="psum", bufs=4, space="PSUM") as psump,
        ):
            xt = constp.tile([68, 128], f16)
            nc.sync.dma_start(xt[:], xt_d[:])
            # warmup matmul to kick the PE out of its low-power pstate early
            # (read back by a cheap DVE op so the psum ring buffer recycles)
            warm = psump.tile([128, SUPER], f32, tag="ps")
            nc.tensor.matmul(warm[:, 0:128], xt[0:4, :], xt[0:4, :])
            wdump = workp.tile([128, 128], f32, tag="wd")
            nc.vector.tensor_scalar_mul(wdump[:], warm[:, 0:128], 0.0)
            rt = constp.tile([68, packw], f16)
            half_w = (nsuper // 2) * SUPER
            for csl in (slice(0, half_w), slice(half_w, packw)):
                nc.sync.dma_start(rt[0:4, csl], rt_d[0:4, csl])
                nc.sync.dma_start(rt[32:36, csl], rt_d[4:8, csl])
                nc.sync.dma_start(rt[64:68, csl], rt_d[8:12, csl])
            off = constp.tile([1, max(nitems, 1)], i32)
            nc.sync.dma_start(off[:], off_d[:])
            colb = constp.tile([128, 3 * packw], bf16)
            for s in range(nsuper):
                sl3 = slice(3 * SUPER * s, 3 * SUPER * (s + 1))
                nc.sync.dma_start(colb[:, sl3], colb_d[:, sl3])

            vint = constp.tile([128, 3 * packw], bf16)
            acc = constp.tile([128, 3 * G], bf16)
            nc.gpsimd.memset(acc[:], 0.0)

            acc3 = acc[:].rearrange("p (c g) -> p c g", c=3)
            vint4 = vint[:].rearrange("p (s c g) -> p s c g", s=nsuper, c=3)

            # register-load scatter offsets up front, batched within item
            # class so the ds bound is per-class; all scatters on DVE
            # (Pool fails codegen on dynamic-offset tensor_tensor)
            vals_all = []
            n_on_v = nitems
            kbase = 0
            for cw, cnt_cls in zip(ITEM_CLASSES, class_counts):
                done = 0
                while done < cnt_cls:
                    cnt = min(SCATTER_BATCH, cnt_cls - done)
                    if kbase < n_on_v:
                        cnt = min(cnt, n_on_v - kbase)
                    eng = nc.vector if kbase < n_on_v else nc.gpsimd
                    _, vals = nc.values_load_multi_w_load_instructions(
                        off[0:1, kbase:kbase + cnt],
                        engines=[eng.engine],
                        min_val=0,
                        max_val=G - cw,
                        skip_runtime_bounds_check=True,
                    )
                    vals_all.extend(vals)
                    done += cnt
                    kbase += cnt
            assert kbase == nitems

            sbase = (packw - D0 * G) // SUPER

            def slot3(d):
                return vint4[:, sbase + d, :, :]

            taccs = constp.tile([128, 3 * G], bf16)
            taccs3 = taccs[:].rearrange("p (c g) -> p c g", c=3)

            next_item = [0]

            def emit_ready_scatters(upto_pos):
                k = next_item[0]
                while (k < nitems
                       and geom[k][1] + geom[k][0] <= upto_pos):
                    cw, ppos = geom[k]
                    s = ppos // SUPER
                    u = ppos - s * SUPER
                    src = vint4[:, s, :, u:u + cw]
                    dst = acc3[:, :, bass.ds(vals_all[k], cw)]
                    eng = nc.vector if k < n_on_v else nc.gpsimd
                    eng.tensor_tensor(dst, dst, src, op=OP.min)
                    k += 1
                next_item[0] = k

            for s in range(nsuper):
                pal = psump.tile([128, SUPER], f32, tag="ps")
                pa = psump.tile([128, SUPER], f32, tag="ps")
                pb = psump.tile([128, SUPER], f32, tag="ps")
                for h in (0, 1):
                    ho = slice(512 * h, 512 * (h + 1))
                    hi = slice(SUPER * s + 512 * h, SUPER * s + 512 * (h + 1))
                    nc.tensor.matmul(pal[:, ho], xt[0:4, :], rt[0:4, hi])
                    nc.tensor.matmul(pa[:, ho], xt[32:36, :], rt[32:36, hi])
                    nc.tensor.matmul(pb[:, ho], xt[64:68, :], rt[64:68, hi])

                m = workp.tile([128, SUPER], bf16, tag="m")
                o = workp.tile([128, SUPER], bf16, tag="o")
                so = workp.tile([128, SUPER], bf16, tag="so")
                sb = workp.tile([128, SUPER], bf16, tag="sb")
                d2 = workp.tile([128, SUPER], bf16, tag="d2")
                dd = workp.tile([128, SUPER], bf16, tag="dd")
                nd = workp.tile([128, SUPER], bf16, tag="nd")

                # o = max(a-L, -a, 0) = max(pal, relu(-pa)); d2 = o^2 + b^2
                # (PSUM readable by ACT/DVE only, max one PSUM input per op)
                nc.scalar.activation(m[:], pa[:], AF.Relu, scale=-1.0)
                nc.vector.scalar_tensor_tensor(
                    o[:], pal[:], 0.0, m[:], op0=OP.bypass, op1=OP.max)
                nc.scalar.activation(so[:], o[:], AF.Square)
                nc.scalar.activation(sb[:], pb[:], AF.Square)
                nc.vector.tensor_tensor(d2[:], so[:], sb[:], op=OP.add)
                nc.scalar.activation(dd[:], d2[:], AF.Sqrt)
                nc.vector.tensor_scalar_add(nd[:], dd[:], -1.0)
                has_items = next_item[0] < nitems and \
                    geom[next_item[0]][1] < SUPER * (s + 1)
                for c in range(3):
                    csl = slice(3 * SUPER * s + c * SUPER,
                                3 * SUPER * s + (c + 1) * SUPER)
                    # scatters/merge-tail wait on all three planes: keep
                    # item superchunks and the last superchunk all-DVE
                    # (gpsimd's mult is ~2.5us)
                    eng = nc.gpsimd if (c == 2 and not has_items
                                        and s != nsuper - 1) else nc.vector
                    eng.tensor_tensor(vint[:, csl], nd[:],
                                      colb[:, csl], op=OP.mult)

                emit_ready_scatters(SUPER * (s + 1))

                # rolling slot merges, quarter-granular (pipelined with
                # later superchunks, finer tail); gpsimd absorbs some
                d = s - sbase
                if d >= 1:
                    for q in range(4):
                        qsl = (slice(None), slice(None),
                               slice(256 * q, 256 * (q + 1)))
                        if d == 1:
                            nc.vector.tensor_tensor(
                                taccs3[qsl], slot3(0)[qsl],
                                slot3(1)[qsl], op=OP.min)
                        else:
                            nc.vector.tensor_tensor(
                                taccs3[qsl], taccs3[qsl],
                                slot3(d)[qsl], op=OP.min)

            # combine scatter acc with rolled slot merge, negate, store
            outst = constp.tile([128, 3 * G], f32)
            NPIECE = 4
            for piece in range(NPIECE):
                slp = slice(piece * 3 * G // NPIECE,
                            (piece + 1) * 3 * G // NPIECE)
                nc.vector.tensor_tensor(acc[:, slp], acc[:, slp],
                                        taccs[:, slp], op=OP.min)
                nc.scalar.activation(outst[:, slp], acc[:, slp],
                                     AF.Relu, scale=-1.0)
                nc.sync.dma_start(out_d[:, slp], outst[:, slp])

    nc.compile()
    return nc


# ---------------------------------------------------------------- entry

def _prepare(strokes, thicknesses, colors):
    wins_by_block, t, col = _build_blocks(strokes, thicknesses, colors)
    pairs = _pair_blocks(wins_by_block)
    packed = []
    counts = []
    for c in range(N_CORES):
        wins = [w for b in pairs[c] for w in wins_by_block[b]]
        slots, excess_cols = _sweep_pack(wins)
        items = _build_strips(excess_cols)
        packed.append((wins, slots, items))
        counts.append([sum(1 for it in items if it[1] == cw)
                       for cw in ITEM_CLASSES])
    class_counts = tuple(max(c[k] for c in counts)
                         for k in range(len(ITEM_CLASSES)))
    in_maps = []
    packw = None
    for c in range(N_CORES):
        wins, slots, items = packed[c]
        tabs, pw = _build_tables(wins, slots, items, class_counts, t, col,
                                 pairs[c])
        assert packw is None or packw == pw
        packw = pw
        in_maps.append(tabs)
    return pairs, in_maps, class_counts, packw


def kernel(strokes, thicknesses, colors):
    _install_ntff_hook()
    from concourse.bass_utils import run_bass_kernel_spmd

    strokes = np.asarray(strokes)
    thicknesses = np.asarray(thicknesses)
    colors = np.asarray(colors)

    pairs, in_maps, class_counts, packw = _prepare(
        strokes, thicknesses, colors)
    key = (class_counts, packw)
    if key not in _PROG_CACHE:
        _PROG_CACHE[key] = _build_program(class_counts, packw)
    nc = _PROG_CACHE[key]

    res = run_bass_kernel_spmd(nc, in_maps, list(range(N_CORES)))

    out = np.zeros((3, G, G), np.float32)
    for c in range(N_CORES):
        o = res.results[c]["out"]                  # (128, 3*G) planar
        for half, b in enumerate(pairs[c]):
            rows = o[half * BH:(half + 1) * BH]    # (64, 3*G)
            for ch in range(3):
                out[ch, BH * b:BH * (b + 1), :] = \
                    rows[:, ch * G:(ch + 1) * G]
    return out


if __name__ == "__main__":
    rng = np.random.default_rng(0)
    s = rng.random((N, 2, 4), np.float32)
    th = rng.random((N, 1), np.float32)
    co = rng.random((N, 3), np.float32)
    g = kernel(s, th, co)
    print("out", g.shape, g.dtype, g.min(), g.max())
